# revision 21
# baseline (speedup 1.0000x reference)
"""Trainium2 Bass kernel for a dense transformer block (self-attn + cross-attn + MLP).

Sharding: 8 cores = 4 batches x 2 query-halves. Host permutes tokens per core so
the core's 512 query tokens are local columns 0-511; causal zig-zag chunk
assignment (chunks {0,3} vs {1,2} of 256 tokens) balances attention work, and a
single uniform SPMD program runs on all cores (dummy kv slots masked via
per-core bias data).

Fast path (used when padding_mask is empty, all linear biases are zero, LN
weights are 1/0, and the pocket context has no all-zero rows — which is what
setup_inputs produces): bf16 residual stream (halves x DMA + speeds LN-stats
matmuls), per-chunk DMA splitting (parallel queues), LN statistics pipelined
into the preceding projection/DMA phase, pocket K/V projections run during
the startup x-DMA window, deferred V-projection chunks fill self-attention
qc0 exp-bound gaps, deferred qcT / split WCO projections fill cross-attention
gaps, slot-paired exps ([128,512] per ACT op with pair-uniform mask bias),
softmax denominators via reciprocal_approx_fast, LN rstd via
Exp(-0.5*Ln(var+eps)) so the whole kernel (except GELU) runs from one ACT
table, and bf16 LN normalize (mean/rstd broadcast copied to bf16 once).

General fallback: the original (slower) build, kept verbatim below.
"""
import os
import numpy as np

B, T, Tp, C, H = 4, 1024, 256, 1024, 16
D = C // H           # 64
KC = C // 128        # 8
FF = 4 * C
FKC = FF // 128      # 32
TMY = 512
EPS = 1e-5
NEG = -120000.0      # pre-scale mask; *0.125 = -15000 -> exp == 0
NEGB = -15000.0      # post-scale mask (activation bias)
SCALE = 0.125

_CACHED = {}

# self-attn kv slots per q-chunk: (chunk, kind) kind: 0=full, 1=diag0, 2=diag1
SLOTS = {0: [(4, 0), (5, 0), (0, 1), (1, 2)],
         1: [(0, 0), (1, 0), (4, 0), (5, 0), (6, 0), (7, 0), (2, 1), (3, 2)]}


def _patch_act_tables():
    """Keep Exp/Ln only in the natural_log_exp_and_others set so the
    activation-table placement pass picks the one table containing both —
    the LN tail chain (…Ln→Exp…) and the attention Exps then share a
    single resident table instead of thrashing 1.3us ACT_TABLE_LOADs.
    Removing functions from the other sets is a safe subset-lie: the sets
    keep their names/indices, and every load the pass emits still loads a
    table that really contains the functions used."""
    import concourse.bacc as bacc_mod
    import concourse.hw_specs as hw
    import concourse.mybir as mybir
    if getattr(bacc_mod, "_ant_act_tables_patched", False):
        return
    orig = hw.get_activation_tables
    AF = mybir.ActivationFunctionType

    def patched(arch):
        tabs = {k: set(v) for k, v in orig(arch).items()}
        for name, s in tabs.items():
            if name != "natural_log_exp_and_others":
                s.discard(AF.Exp)
                s.discard(AF.Ln)
        return tabs

    bacc_mod.get_activation_tables = patched
    bacc_mod._ant_act_tables_patched = True


def _build_nc_fast(debug=False):
    import concourse.mybir as mybir
    from concourse import bacc
    from concourse.tile import TileContext

    _patch_act_tables()

    F32 = mybir.dt.float32
    F32R = mybir.dt.float32r
    BF16 = mybir.dt.bfloat16
    AF = mybir.ActivationFunctionType
    ALU = mybir.AluOpType

    nc = bacc.Bacc("TRN2", target_bir_lowering=False, debug=False)

    def din(name, shape, dt):
        return nc.declare_dram_parameter(name, list(shape), dt, isOutput=False)

    # packed inputs: [128, kc*cols] slab-major
    XMY = din("XMY", [128, KC * TMY], BF16)
    XT2 = din("XT2", [128, KC * TMY], BF16)
    PKT = din("PKT", [128, KC * Tp], BF16)
    # weights, slab-packed: slab g = [128, nk*512] contiguous
    WQKVF = din("WQKVF", [128, 6 * KC * 512], BF16)
    WOF = din("WOF", [128, 2 * KC * 512], BF16)
    WQF = din("WQF", [128, 2 * KC * 512], BF16)
    WKVF = din("WKVF", [128, 4 * KC * 512], BF16)
    WCOF = din("WCOF", [128, 2 * KC * 512], BF16)
    WFCF = din("WFCF", [128, 8 * KC * 512], BF16)
    WFPF = din("WFPF", [128, 8 * FKC * 128], BF16)
    TRI01 = din("TRI01", [128, 128], BF16)
    ONL8 = din("ONL8", [128, 8], BF16)
    ON8B = din("ON8B", [8, 128], BF16)
    EPS8 = din("EPS8", [8, 1], F32)
    PADS2 = din("PADS2", [128, 8], F32)
    OUT2 = nc.declare_dram_parameter("OUT2", [128, KC * TMY], BF16, isOutput=True)

    uid = [0]

    def nm(p):
        uid[0] += 1
        return f"{p}_{uid[0]}"

    with TileContext(nc) as tc:
        with tc.tile_pool(name="cst", bufs=1) as cst, \
             tc.tile_pool(name="act", bufs=1) as act, \
             tc.tile_pool(name="wp", bufs=3) as wp, \
             tc.tile_pool(name="tmp", bufs=3) as tmp, \
             tc.tile_pool(name="ex", bufs=6) as exp_pool, \
             tc.tile_pool(name="ps", bufs=4, space="PSUM") as ps:

            # Warm up custom-DVE uop table + gpsimd library BEFORE any tile
            # DMA lands: the table/library staging at kernel start writes
            # SBUF and races with concurrently-landing input DMAs (observed
            # as a +-2.0 constant pattern over early tiles). Every
            # DMA-loaded tile gets a 1-element guard write that depends on
            # the warm-up, ordering its load after the staging completes.
            warm3 = cst.tile([128, 4], F32, tag="warm3")
            nc.vector.memset(warm3, 1.0)

            def guard(ap):
                with nc.allow_low_precision(reason="dma order guard"):
                    nc.vector.tensor_copy(ap, warm3[0:1, 0:1])

            def c_tile(name, dram, shape, dt):
                t = cst.tile(list(shape), dt, tag=name)
                guard(t[0:1, 0:1])
                nc.sync.dma_start(out=t, in_=dram[:, :])
                return t

            tri01 = c_tile("tri01", TRI01, [128, 128], BF16)
            onl8 = c_tile("onl8", ONL8, [128, 8], BF16)
            on8b = c_tile("on8b", ON8B, [8, 128], BF16)
            eps8 = c_tile("eps8", EPS8, [8, 1], F32)
            pads2 = c_tile("pads2", PADS2, [128, 8], F32)

            def wslab(Wf, g, nk=KC, ncols=512, twod=False, tag="wslab",
                      bufs=None):
                t = wp.tile([128, nk, ncols], BF16, tag=tag, name=nm("ws"),
                            bufs=bufs)
                guard(t[0:1, 0, 0:1])
                off = g * nk * ncols
                if twod:
                    for k in range(nk):
                        nc.sync.dma_start(
                            out=t[:, k, :],
                            in_=Wf[:, off + k * ncols:off + (k + 1) * ncols])
                else:
                    nc.sync.dma_start(
                        out=t,
                        in_=Wf[:, off:off + nk * ncols].rearrange(
                            "p (k c) -> p k c", c=ncols))
                return t

            # ---- residual stream x: bf16, chunk-pair DMAs (2KB contiguous
            # rows per partition — fat descriptors — while still letting the
            # LN1 stats start on early chunks). Emission order (= queue
            # order) puts xmy first, then pocket inputs/WKV slabs
            # interleaved with xt2.
            def chunked(dst, src, w, step=2):
                for k in range(0, KC, step):
                    guard(dst[0:1, k, 0:1])
                    nc.sync.dma_start(
                        out=dst[:, k:k + step, :],
                        in_=src[:, k * w:(k + step) * w].rearrange(
                            "p (k c) -> p k c", c=w))

            xmy = act.tile([128, KC, TMY], BF16, tag="res", bufs=2, name="xmy")
            chunked(xmy, XMY, TMY)
            pkT = act.tile([128, KC, Tp], BF16, tag="pkT", bufs=1)
            chunked(pkT, PKT, Tp, step=4)
            kvs = {}
            kvs[0] = wslab(WKVF, 0, tag="kvw", bufs=2)
            xt2 = act.tile([128, KC, TMY], BF16, tag="res", bufs=2, name="xt2")
            chunked(xt2, XT2, TMY)
            kvs[1] = wslab(WKVF, 1, tag="kvw", bufs=2)

            def ln_stats_unit(xs, pm, ps2, k):
                """Accumulate sum(x) / sum(x^2) for one [128,512] chunk.
                pm's matmul goes first (it doesn't wait on the square)."""
                nc.tensor.matmul(pm, onl8, xs,
                                 start=(k == 0), stop=(k == KC - 1))
                sq = tmp.tile([128, 512], BF16, tag="lnsq", bufs=3, name=nm("sq"))
                nc.vector.tensor_mul(sq, xs, xs)
                nc.tensor.matmul(ps2, onl8, sq,
                                 start=(k == 0), stop=(k == KC - 1))

            def ln_tail(pm, ps2):
                """Serial mean/rstd chain; returns bf16 broadcast tiles."""
                m8 = tmp.tile([8, 512], BF16, tag="lnm8", bufs=2, name=nm("m8"))
                nc.scalar.activation(m8, pm, AF.Copy, scale=1.0 / C)
                m2 = tmp.tile([8, 512], F32, tag="lnf", bufs=3, name=nm("m2"))
                nc.scalar.activation(m2, pm, AF.Square, scale=1.0 / C)
                var8 = tmp.tile([8, 512], F32, tag="lnf", bufs=3, name=nm("var8"))
                nc.vector.scalar_tensor_tensor(
                    out=var8, in0=ps2, scalar=1.0 / C, in1=m2,
                    op0=ALU.mult, op1=ALU.subtract)
                l8 = tmp.tile([8, 512], F32, tag="lnf", bufs=3, name=nm("l8"))
                nc.scalar.activation(l8, var8, AF.Ln, bias=eps8[:, :])
                rs8 = tmp.tile([8, 512], BF16, tag="lnm8", bufs=2, name=nm("rs8"))
                nc.scalar.activation(rs8, l8, AF.Exp, scale=-0.5)
                pmb = ps.tile([128, 512], F32, tag="mm", name=nm("pmb"))
                nc.tensor.matmul(pmb, on8b, m8, start=True, stop=True)
                prb = ps.tile([128, 512], F32, tag="mm", name=nm("prb"))
                nc.tensor.matmul(prb, on8b, rs8, start=True, stop=True)
                pmbb = tmp.tile([128, 512], BF16, tag="lnbb", bufs=4,
                                name=nm("pmbb"))
                nc.vector.tensor_copy(pmbb, pmb)
                prbb = tmp.tile([128, 512], BF16, tag="lnbb", bufs=4,
                                name=nm("prbb"))
                nc.vector.tensor_copy(prbb, prb)
                return pmbb, prbb

            def ln_norm(xs_fn, pmbb, prbb, out_t, nt):
                for k in range(KC):
                    t = tmp.tile([128, 512], BF16, tag="lnsq", bufs=3,
                                 name=nm("lt"))
                    nc.vector.tensor_sub(t, xs_fn(k), pmbb)
                    nc.vector.tensor_mul(out_t[:, k, 512 * nt:512 * (nt + 1)],
                                         t, prbb)

            def attention(kTt, vaugt, q_t, slots_fn, pad_col_fn, out_t, masked,
                          fillers=None):
                """out_t [128, KC, TMY] bf16; slot-paired exps. fillers maps
                qc -> list of closures (independent PE work); one is emitted
                per head-pair to keep the PE dense through the exp-bound
                stretches, with leftovers flushed before the next qc."""
                fillers = fillers or {}
                for qc in range(2):
                    flist = fillers.get(qc, [])
                    slots = slots_fn(qc)
                    npair = len(slots) // 2
                    for hp in range(8):
                        if flist:
                            flist.pop(0)()
                        pys = []
                        for h in (2 * hp, 2 * hp + 1):
                            hb = 64 * (h % 2)
                            py = ps.tile([65, 256], F32, tag="acc", bufs=4,
                                         name=nm("py"))
                            for pi in range(npair):
                                psc2 = ps.tile([128, 512], F32, tag="mm",
                                               name=nm("sc"))
                                for j in range(2):
                                    ck, kind = slots[2 * pi + j]
                                    nc.tensor.matmul(
                                        psc2[:, 256 * j:256 * (j + 1)],
                                        kTt[hb:hb + 64, h // 2,
                                            128 * ck:128 * (ck + 1)],
                                        q_t[hb:hb + 64, h // 2,
                                            256 * qc:256 * (qc + 1)],
                                        start=True, stop=True)
                                ex2 = exp_pool.tile([128, 512], BF16, tag="ex")
                                col = pad_col_fn(qc, pi)
                                nc.scalar.activation(
                                    ex2, psc2, AF.Exp,
                                    bias=pads2[:, col:col + 1], scale=SCALE)
                                for j in range(2):
                                    ck, kind = slots[2 * pi + j]
                                    if masked and kind == 1:
                                        nc.vector.tensor_mul(
                                            ex2[:, 256 * j:256 * j + 128],
                                            ex2[:, 256 * j:256 * j + 128], tri01)
                                    elif masked and kind == 2:
                                        nc.vector.memset(
                                            ex2[:, 256 * j:256 * j + 128], 0.0)
                                        nc.vector.tensor_mul(
                                            ex2[:, 256 * j + 128:256 * (j + 1)],
                                            ex2[:, 256 * j + 128:256 * (j + 1)],
                                            tri01)
                                for j in range(2):
                                    ck, _ = slots[2 * pi + j]
                                    nc.tensor.matmul(
                                        py,
                                        vaugt[:, ck, 65 * h:65 * h + 65],
                                        ex2[:, 256 * j:256 * (j + 1)],
                                        start=(pi == 0 and j == 0),
                                        stop=(pi == npair - 1 and j == 1))
                            pys.append(py)
                        for j, h in enumerate((2 * hp, 2 * hp + 1)):
                            hb = 64 * (h % 2)
                            n1 = tmp.tile([1, 256], F32, tag="n1", bufs=2,
                                          name=nm("n1"))
                            nc.vector.tensor_copy(n1, pys[j][64:65, :])
                            rec1 = tmp.tile([1, 256], F32, tag="rec1", bufs=2,
                                            name=nm("rc"))
                            nc.vector.reciprocal_approx_fast(rec1, n1)
                            rsb = tmp.tile([64, 256], F32, tag="rsb", bufs=2,
                                           name=nm("rb"))
                            nc.gpsimd.partition_broadcast(rsb, rec1, channels=64)
                            nc.vector.tensor_mul(
                                out_t[hb:hb + 64, h // 2,
                                      256 * qc:256 * (qc + 1)],
                                pys[j][0:64, :], rsb)
                    while flist:
                        flist.pop(0)()

            # ================ phase 1: LN1 stats (DMA-paced) + pocket K =====
            kcT = act.tile([128, KC, Tp], BF16, tag="kcT", bufs=1)

            def kc_unit(g, ml):
                m = 4 * g + ml
                p = ps.tile([128, 512], F32, tag="mm", name=nm("kc"))[:, :Tp]
                for k in range(KC):
                    nc.tensor.matmul(
                        p, kvs[g][:, k, 128 * ml:128 * (ml + 1)],
                        pkT[:, k, :], start=(k == 0), stop=(k == KC - 1))
                nc.vector.tensor_copy(kcT[:, m, :], p)

            pm0 = ps.tile([8, 512], F32, tag="acc", name="pm0")
            ps20 = ps.tile([8, 512], F32, tag="acc", name="ps20")
            pm1 = ps.tile([8, 512], F32, tag="acc", name="pm1")
            ps21 = ps.tile([8, 512], F32, tag="acc", name="ps21")
            for k in range(KC):
                ln_stats_unit(xmy[:, k, :], pm0, ps20, k)
            for ml in range(4):
                kc_unit(0, ml)
            for k in range(KC):
                ln_stats_unit(xt2[:, k, :], pm1, ps21, k)

            kvs[2] = wslab(WKVF, 2, tag="kvw", bufs=2)
            kvs[3] = wslab(WKVF, 3, tag="kvw", bufs=2)

            # pocket K second half + pocket V surround the LN1 tails so the
            # PE stays busy through the serial mean/rstd chains and the
            # a1 normalize stretch.
            kc_unit(1, 0)
            kc_unit(1, 1)
            pmbb0, prbb0 = ln_tail(pm0, ps20)
            kc_unit(1, 2)
            kc_unit(1, 3)
            pmbb1, prbb1 = ln_tail(pm1, ps21)

            vcaug = act.tile([128, 2, 16 * 65], BF16, tag="vcaug", bufs=1)
            for t2 in range(2):
                nc.vector.memset(
                    vcaug.rearrange("p c (h q) -> p c h q", q=65)[:, t2, :, 64:65],
                    1.0)

            def vc_unit(g, t2, half):
                def f():
                    p = ps.tile([128, 512], F32, tag="mm",
                                name=nm("vc"))[:, 256 * half:256 * (half + 1)]
                    for k in range(KC):
                        nc.tensor.matmul(
                            p, pkT[:, k, 128 * t2:128 * (t2 + 1)],
                            kvs[2 + g][:, k, 256 * half:256 * (half + 1)],
                            start=(k == 0), stop=(k == KC - 1))
                    nc.scalar.copy(
                        vcaug.rearrange("p c (h q) -> p c h q", q=65)[
                            :, t2, 8 * g + 4 * half:8 * g + 4 * (half + 1), 0:64],
                        p.rearrange("p (h q) -> p h q", q=64))
                return f

            vc_all = [vc_unit(g, t2, half)
                      for g in range(2) for t2 in range(2)
                      for half in range(2)]
            # three vc halves cover the a1-normalize stretch; the rest fill
            # self-attention qc1's exp-bound gaps
            for f in vc_all[:3]:
                f()
            vc_deferred = vc_all[3:]

            a1 = act.tile([128, KC, T], BF16, tag="big", bufs=2, name="a1")
            ln_norm(lambda k: xmy[:, k, :], pmbb0, prbb0, a1, 0)
            ln_norm(lambda k: xt2[:, k, :], pmbb1, prbb1, a1, 1)

            # ================ phase 2: QKV projections ================
            qT = act.tile([128, KC, TMY], BF16, tag="bb", bufs=2, name="qT")
            for g in range(2):
                slab = wslab(WQKVF, g)
                for ml in range(4):
                    m = 4 * g + ml
                    p = ps.tile([128, 512], F32, tag="mm", name=nm("q"))
                    for k in range(KC):
                        nc.tensor.matmul(p, slab[:, k, 128 * ml:128 * (ml + 1)],
                                         a1[:, k, 0:TMY],
                                         start=(k == 0), stop=(k == KC - 1))
                    nc.scalar.copy(qT[:, m, :], p)

            kT = act.tile([128, KC, T], BF16, tag="big", bufs=2, name="kT")
            for g in range(2):
                slab = wslab(WQKVF, 2 + g)
                for ml in range(4):
                    m = 4 * g + ml
                    for nt in range(2):
                        p = ps.tile([128, 512], F32, tag="mm", name=nm("k"))
                        for k in range(KC):
                            nc.tensor.matmul(
                                p, slab[:, k, 128 * ml:128 * (ml + 1)],
                                a1[:, k, 512 * nt:512 * (nt + 1)],
                                start=(k == 0), stop=(k == KC - 1))
                        nc.vector.tensor_copy(kT[:, m, 512 * nt:512 * (nt + 1)], p)

            vaug = act.tile([128, KC, 16 * 65], BF16, tag="vaug", bufs=1)
            for t8 in range(KC):
                nc.vector.memset(
                    vaug.rearrange("p c (h q) -> p c h q", q=65)[:, t8, :, 64:65],
                    1.0)
            vslab = [wslab(WQKVF, 4), wslab(WQKVF, 5)]

            def v_unit(g, t8):
                def f():
                    p = ps.tile([128, 512], F32, tag="mm", name=nm("v"))
                    for k in range(KC):
                        nc.tensor.matmul(p, a1[:, k, 128 * t8:128 * (t8 + 1)],
                                         vslab[g][:, k, :],
                                         start=(k == 0), stop=(k == KC - 1))
                    nc.scalar.copy(
                        vaug.rearrange("p c (h q) -> p c h q", q=65)[
                            :, t8, 8 * g:8 * (g + 1), 0:64],
                        p.rearrange("p (h q) -> p h q", q=64))
                return f

            # V for the kv chunks qc0 touches; chunks {2,3,6,7} (only needed
            # from qc1) are deferred into qc0's exp-bound stretches.
            for t8 in (0, 1, 4, 5):
                for g in range(2):
                    v_unit(g, t8)()
            v_deferred = [v_unit(g, t8) for t8 in (2, 3, 6, 7) for g in range(2)]

            # ================ phase 3: self-attention ================
            yT = act.tile([128, KC, TMY], BF16, tag="bb", bufs=2, name="yT")
            attention(kT, vaug, qT, lambda qc: SLOTS[qc],
                      lambda qc, pi: (0 if qc == 0 else 2) + pi, yT, masked=True,
                      fillers={0: v_deferred, 1: vc_deferred})

            # ================ phase 4: out-proj + residual + LNc stats ======
            xr1 = act.tile([128, KC, TMY], BF16, tag="res", bufs=2, name="xr1")
            pmc = ps.tile([8, 512], F32, tag="acc", name="pmc")
            ps2c = ps.tile([8, 512], F32, tag="acc", name="ps2c")
            for g in range(2):
                slab = wslab(WOF, g)
                for ml in range(4):
                    m = 4 * g + ml
                    p = ps.tile([128, 512], F32, tag="mm", name=nm("o"))
                    for k in range(KC):
                        nc.tensor.matmul(p, slab[:, k, 128 * ml:128 * (ml + 1)],
                                         yT[:, k, :],
                                         start=(k == 0), stop=(k == KC - 1))
                    nc.vector.tensor_add(xr1[:, m, :], p, xmy[:, m, :])
                    ln_stats_unit(xr1[:, m, :], pmc, ps2c, m)

            # ================ phase 5: cross-attention ================
            pmbbc, prbbc = ln_tail(pmc, ps2c)
            a2 = act.tile([128, KC, TMY], BF16, tag="bb", bufs=2, name="a2")
            ln_norm(lambda k: xr1[:, k, :], pmbbc, prbbc, a2, 0)

            qcT = act.tile([128, KC, TMY], BF16, tag="bb", bufs=2, name="qcT")
            qslab = [wslab(WQF, 0), wslab(WQF, 1)]

            def qc_unit(g, ml):
                def f():
                    m = 4 * g + ml
                    p = ps.tile([128, 512], F32, tag="mm", name=nm("cq"))
                    for k in range(KC):
                        nc.tensor.matmul(p, qslab[g][:, k, 128 * ml:128 * (ml + 1)],
                                         a2[:, k, :],
                                         start=(k == 0), stop=(k == KC - 1))
                    nc.scalar.copy(qcT[:, m, :], p)
                return f

            qc_unit(0, 0)()
            q_deferred = [qc_unit(m // 4, m % 4) for m in range(1, KC)]

            # WCO projection of qc0's columns runs as qc1 fillers
            ycT = act.tile([128, KC, TMY], BF16, tag="bb", bufs=2, name="ycT")
            xr2 = act.tile([128, KC, TMY], BF16, tag="res", bufs=2, name="xr2")
            coslab = [wslab(WCOF, 0), wslab(WCOF, 1)]

            def wco_half0(m):
                def f():
                    p = ps.tile([128, 512], F32, tag="mm", name=nm("co"))[:, :256]
                    for k in range(KC):
                        nc.tensor.matmul(
                            p, coslab[m // 4][:, k, 128 * (m % 4):128 * (m % 4 + 1)],
                            ycT[:, k, 0:256],
                            start=(k == 0), stop=(k == KC - 1))
                    nc.vector.tensor_add(xr2[:, m, 0:256], p, xr1[:, m, 0:256])
                return f

            attention(kcT, vcaug, qcT, lambda qc: [(0, 0), (1, 0)],
                      lambda qc, pi: 6, ycT, masked=False,
                      fillers={0: q_deferred,
                               1: [wco_half0(m) for m in range(KC)]})

            # ================ phase 6: WCO half 1 + residual + LN2 stats ====
            pm2t = ps.tile([8, 512], F32, tag="acc", name="pm2t")
            ps22t = ps.tile([8, 512], F32, tag="acc", name="ps22t")
            for m in range(KC):
                p = ps.tile([128, 512], F32, tag="mm", name=nm("co2"))[:, :256]
                for k in range(KC):
                    nc.tensor.matmul(
                        p, coslab[m // 4][:, k, 128 * (m % 4):128 * (m % 4 + 1)],
                        ycT[:, k, 256:512],
                        start=(k == 0), stop=(k == KC - 1))
                nc.vector.tensor_add(xr2[:, m, 256:512], p, xr1[:, m, 256:512])
                ln_stats_unit(xr2[:, m, :], pm2t, ps22t, m)

            # ================ phase 7: MLP ================
            pmbb2, prbb2 = ln_tail(pm2t, ps22t)
            a3 = act.tile([128, KC, TMY], BF16, tag="bb", bufs=2, name="a3")
            ln_norm(lambda k: xr2[:, k, :], pmbb2, prbb2, a3, 0)

            hT = act.tile([128, FKC, TMY], BF16, tag="hT", bufs=1)
            for g in range(8):
                slab = wslab(WFCF, g)
                for ml in range(4):
                    m = 4 * g + ml
                    p = ps.tile([128, 512], F32, tag="mm", name=nm("fc"))
                    for k in range(KC):
                        nc.tensor.matmul(p, slab[:, k, 128 * ml:128 * (ml + 1)],
                                         a3[:, k, :],
                                         start=(k == 0), stop=(k == KC - 1))
                    nc.scalar.activation(hT[:, m, :], p, AF.Gelu)

            outT = act.tile([128, KC, TMY], BF16, tag="res", bufs=2, name="outT")
            for m in range(KC):
                slab = wslab(WFPF, m, nk=FKC, ncols=128)
                pout = ps.tile([128, 512], F32, tag="acc", bufs=4, name=nm("fp"))
                for k in range(FKC):
                    nc.tensor.matmul(pout, slab[:, k, :], hT[:, k, :],
                                     start=(k == 0), stop=(k == FKC - 1))
                nc.vector.tensor_add(outT[:, m, :], pout, xr2[:, m, :])
                nc.sync.dma_start(out=OUT2[:, m * TMY:(m + 1) * TMY],
                                  in_=outT[:, m, :])

    nc.compile()
    return nc


def _slabify(W, slab_cols=512):
    """[K, N] -> [128, (N//slab_cols) * (K//128) * slab_cols] slab-major."""
    K, N = W.shape
    kc = K // 128
    G = N // slab_cols
    Wr = W.reshape(kc, 128, G, slab_cols).transpose(1, 2, 0, 3)
    return np.ascontiguousarray(Wr.reshape(128, G * kc * slab_cols))


def _pack_tok(xT, lo, hi):
    """[C, T] fp32 -> [128, KC*(hi-lo)] k-major."""
    w = hi - lo
    return np.ascontiguousarray(
        xT.reshape(KC, 128, -1)[:, :, lo:hi].transpose(1, 0, 2).reshape(128, KC * w))


def _host_prepare_fast(inputs):
    import ml_dtypes
    BF = ml_dtypes.bfloat16

    shared = {
        "WQKVF": _slabify(np.asarray(inputs["Wqkv"], np.float32)).astype(BF),
        "WOF": _slabify(np.asarray(inputs["Wo"], np.float32)).astype(BF),
        "WQF": _slabify(np.asarray(inputs["Wq"], np.float32)).astype(BF),
        "WKVF": _slabify(np.asarray(inputs["Wkv"], np.float32)).astype(BF),
        "WCOF": _slabify(np.asarray(inputs["Wco"], np.float32)).astype(BF),
        "WFCF": _slabify(np.asarray(inputs["Wfc"], np.float32)).astype(BF),
        "WFPF": _slabify(np.asarray(inputs["Wfp"], np.float32), 128).astype(BF),
        "TRI01": np.where(np.arange(128)[None, :] >= np.arange(128)[:, None],
                          1.0, 0.0).astype(BF),
        "ONL8": np.ones((128, 8), np.float32).astype(BF),
        "ON8B": np.full((8, 128), 0.125, np.float32).astype(BF),
        "EPS8": np.full((8, 1), EPS, np.float32),
    }

    x = np.asarray(inputs["x"], np.float32)
    pocket = np.asarray(inputs["pocket_context"], np.float32)

    attend = {}
    for half in range(2):
        attend[half] = {0: {0, 1} | ({4, 5} if half == 1 else set()),
                        1: {0, 1, 2, 3, 4, 5} | ({6, 7} if half == 0 else set())}

    maps = []
    for core in range(8):
        b, half = core // 2, core % 2
        chunks = [0, 3, 1, 2] if half == 0 else [1, 2, 0, 3]
        idx = np.concatenate([np.arange(256 * c, 256 * (c + 1)) for c in chunks])
        xT = np.ascontiguousarray(x[b][idx].T)
        pkT = np.ascontiguousarray(pocket[b].T)

        pads2 = np.zeros((128, 8), np.float32)
        for qc in range(2):
            slots = SLOTS[qc]
            for pi in range(len(slots) // 2
                            ):
                ckA, ckB = slots[2 * pi][0], slots[2 * pi + 1][0]
                att_a = ckA in attend[half][qc]
                att_b = ckB in attend[half][qc]
                assert att_a == att_b, "pair attend status must be uniform"
                col = (0 if qc == 0 else 2) + pi
                pads2[:, col] = 0.0 if att_a else NEGB
        # col 6 = cross pair: always attended (no pocket padding on fast path)

        maps.append(dict(shared,
                         XMY=_pack_tok(xT, 0, TMY).astype(BF),
                         XT2=_pack_tok(xT, TMY, T).astype(BF),
                         PKT=_pack_tok(pkT, 0, Tp).astype(BF),
                         PADS2=pads2))
    return maps


def _is_fast(inputs):
    z = lambda name: not np.any(np.asarray(inputs[name]))
    ones = lambda name: np.all(np.asarray(inputs[name]) == 1.0)
    pocket = np.asarray(inputs["pocket_context"], np.float32)
    return (not np.any(np.asarray(inputs["padding_mask"]))
            and np.all(np.abs(pocket).sum(-1) != 0)
            and all(z(n) for n in ("bqkv", "bo", "bq", "bkv", "bco", "bfc", "bfp"))
            and all(z(n) for n in ("ln1_b", "lnc_b", "ln2_b"))
            and all(ones(n) for n in ("ln1_w", "lnc_w", "ln2_w")))


LAST_RESULTS = None


def kernel(**inputs):
    global LAST_RESULTS
    from concourse.bass_utils import run_bass_kernel_spmd

    trace = bool(int(os.environ.get("KERNEL_TRACE", "0")))
    if _is_fast(inputs):
        if "nc_fast" not in _CACHED:
            _CACHED["nc_fast"] = _build_nc_fast()
        nc = _CACHED["nc_fast"]
        in_maps = _host_prepare_fast(inputs)
        res = run_bass_kernel_spmd(nc, in_maps, list(range(8)), trace=trace)
        LAST_RESULTS = res
        out = np.zeros((B, T, C), np.float32)
        for core in range(8):
            b, half = core // 2, core % 2
            chunks = [0, 3] if half == 0 else [1, 2]
            o2 = np.asarray(res.results[core]["OUT2"], np.float32)
            o = o2.reshape(128, KC, TMY).transpose(1, 0, 2).reshape(C, TMY)
            for j, c in enumerate(chunks):
                out[b, 256 * c:256 * (c + 1), :] = o[:, 256 * j:256 * (j + 1)].T
        return out

    # -------- general fallback (original baseline path) --------
    if "nc_gen" not in _CACHED:
        _CACHED["nc_gen"] = _build_nc_general()
    nc = _CACHED["nc_gen"]
    in_maps = [_host_prepare_general(core, inputs) for core in range(8)]
    res = run_bass_kernel_spmd(nc, in_maps, list(range(8)), trace=trace)
    LAST_RESULTS = res
    out = np.zeros((B, T, C), np.float32)
    for core in range(8):
        b, half = core // 2, core % 2
        chunks = [0, 3] if half == 0 else [1, 2]
        o = np.asarray(res.results[core]["OUT"])
        for j, c in enumerate(chunks):
            out[b, 256 * c:256 * (c + 1), :] = o[:, 256 * j:256 * (j + 1)].T
    return out


def _build_nc_general(sim_mode=False):
    import concourse.mybir as mybir
    from concourse import bacc
    from concourse.tile import TileContext

    F32 = mybir.dt.float32
    F32R = mybir.dt.float32r
    BF16 = mybir.dt.bfloat16
    AF = mybir.ActivationFunctionType
    ALU = mybir.AluOpType
    GELU = AF.Tanh if sim_mode else AF.Gelu

    nc = bacc.Bacc("TRN2", target_bir_lowering=False, debug=False)

    def din(name, shape, dt):
        return nc.declare_dram_parameter(name, list(shape), dt, isOutput=False)

    XT = din("XT", [C, T], F32R)
    XMY = din("XMY", [C, TMY], F32R)
    PKT = din("PKT", [C, Tp], BF16)
    WQKV = din("WQKV", [C, 3 * C], BF16)
    WO = din("WO", [C, C], BF16)
    WQ = din("WQ", [C, C], BF16)
    WKV = din("WKV", [C, 2 * C], BF16)
    WCO = din("WCO", [C, C], BF16)
    WFC = din("WFC", [C, FF], BF16)
    WFP = din("WFP", [FF, C], BF16)
    BQK = din("BQK", [128, 16], F32)
    BO8 = din("BO8", [128, KC], F32)
    BQ8 = din("BQ8", [128, KC], F32)
    BKC8 = din("BKC8", [128, KC], F32)
    BCO8 = din("BCO8", [128, KC], F32)
    BFC32 = din("BFC32", [128, FKC], F32)
    BFP8 = din("BFP8", [128, KC], F32)
    LN1W = din("LN1W", [128, KC], F32)
    LN1B = din("LN1B", [128, KC], F32)
    LNCW = din("LNCW", [128, KC], F32)
    LNCB = din("LNCB", [128, KC], F32)
    LN2W = din("LN2W", [128, KC], F32)
    LN2B = din("LN2B", [128, KC], F32)
    T1M = din("T1M", [128, 128], F32)
    M1M = din("M1M", [128, 256], F32)
    E2 = din("E2", [64, 128], BF16)
    ONL8 = din("ONL8", [128, 8], F32R)
    ON8B = din("ON8B", [8, 128], F32R)
    EPS8 = din("EPS8", [8, 1], F32)
    ONES16 = din("ONES16", [128, 16], BF16)
    KEEPBC = din("KEEPBC", [128, TMY], F32)
    VBBC = din("VBBC", [128, C], BF16)
    VCBBC = din("VCBBC", [128, C], BF16)
    PADS = din("PADS", [128, 16], F32)
    PADC = din("PADC", [128, 2], F32)
    OUT = nc.declare_dram_parameter("OUT", [C, TMY], F32, isOutput=True)

    uid = [0]

    def nm(p):
        uid[0] += 1
        return f"{p}_{uid[0]}"

    with TileContext(nc) as tc:
        with tc.tile_pool(name="cst", bufs=1) as cst, \
             tc.tile_pool(name="act", bufs=1) as act, \
             tc.tile_pool(name="wp", bufs=2) as wp, \
             tc.tile_pool(name="tmp", bufs=3) as tmp, \
             tc.tile_pool(name="ex", bufs=6) as exp_pool, \
             tc.tile_pool(name="ps", bufs=4, space="PSUM") as ps:

            def c_tile(name, dram, shape, dt):
                t = cst.tile(list(shape), dt, tag=name)
                nc.sync.dma_start(out=t, in_=dram[:, :])
                return t

            t1m = c_tile("t1m", T1M, [128, 128], F32)
            m1m = c_tile("m1m", M1M, [128, 256], F32)
            e2 = c_tile("e2", E2, [64, 128], BF16)
            onl8 = c_tile("onl8", ONL8, [128, 8], F32R)
            on8b = c_tile("on8b", ON8B, [8, 128], F32R)
            eps8 = c_tile("eps8", EPS8, [8, 1], F32)
            ones16 = c_tile("ones16", ONES16, [128, 16], BF16)
            keepbc = c_tile("keepbc", KEEPBC, [128, TMY], F32)
            vbbc = c_tile("vbbc", VBBC, [128, C], BF16)
            vcbbc = c_tile("vcbbc", VCBBC, [128, C], BF16)
            pads = c_tile("pads", PADS, [128, 16], F32)
            padc = c_tile("padc", PADC, [128, 2], F32)
            bqk = c_tile("bqk", BQK, [128, 16], F32)
            bo8 = c_tile("bo8", BO8, [128, KC], F32)
            bq8 = c_tile("bq8", BQ8, [128, KC], F32)
            bkc8 = c_tile("bkc8", BKC8, [128, KC], F32)
            bco8 = c_tile("bco8", BCO8, [128, KC], F32)
            bfc32 = c_tile("bfc32", BFC32, [128, FKC], F32)
            bfp8 = c_tile("bfp8", BFP8, [128, KC], F32)
            ln1w = c_tile("ln1w", LN1W, [128, KC], F32)
            ln1b = c_tile("ln1b", LN1B, [128, KC], F32)
            lncw = c_tile("lncw", LNCW, [128, KC], F32)
            lncb = c_tile("lncb", LNCB, [128, KC], F32)
            ln2w = c_tile("ln2w", LN2W, [128, KC], F32)
            ln2b = c_tile("ln2b", LN2B, [128, KC], F32)

            xmy = act.tile([128, KC, TMY], F32R, tag="res", bufs=2, name="xmy")
            for k in range(KC):
                nc.sync.dma_start(out=xmy[:, k, :], in_=XMY[128 * k:128 * (k + 1), :])
            pkT = act.tile([128, KC, Tp], BF16, tag="pkT", bufs=1)
            for k in range(KC):
                nc.sync.dma_start(out=pkT[:, k, :], in_=PKT[128 * k:128 * (k + 1), :])

            def layernorm(get_x, W, w_col, b_col, out_t):
                ntile = (W + 511) // 512
                for nt in range(ntile):
                    w0 = nt * 512
                    ww = min(W, w0 + 512) - w0
                    xs = [get_x(k, nt) for k in range(KC)]
                    pm = ps.tile([8, 512], F32, tag="mm", name=nm("pm"))[:, :ww]
                    ps2 = ps.tile([8, 512], F32, tag="mm", name=nm("ps2"))[:, :ww]
                    for k in range(KC):
                        sq = tmp.tile([128, 512], F32R, tag="lnsq", bufs=2, name=nm("sq"))[:, :ww]
                        nc.scalar.activation(sq, xs[k], AF.Square)
                        nc.tensor.matmul(pm, onl8, xs[k],
                                         start=(k == 0), stop=(k == KC - 1))
                        nc.tensor.matmul(ps2, onl8, sq,
                                         start=(k == 0), stop=(k == KC - 1))
                    m8 = tmp.tile([8, 512], F32R, tag="lnm8", bufs=2, name=nm("m8"))[:, :ww]
                    nc.scalar.activation(m8, pm, AF.Copy, scale=1.0 / C)
                    m2 = tmp.tile([8, 512], F32, tag="lnf", bufs=3, name=nm("m2"))[:, :ww]
                    nc.scalar.activation(m2, pm, AF.Square, scale=1.0 / C)
                    var8 = tmp.tile([8, 512], F32, tag="lnf", bufs=3, name=nm("var8"))[:, :ww]
                    nc.vector.scalar_tensor_tensor(
                        out=var8, in0=ps2, scalar=1.0 / C, in1=m2,
                        op0=ALU.mult, op1=ALU.subtract)
                    sd8 = tmp.tile([8, 512], F32, tag="lnf", bufs=3, name=nm("sd8"))[:, :ww]
                    nc.scalar.activation(sd8, var8, AF.Sqrt, bias=eps8[:, :])
                    rs8 = tmp.tile([8, 512], F32R, tag="lnm8", bufs=2, name=nm("rs8"))[:, :ww]
                    with nc.allow_low_precision(reason="ln rstd f32r"):
                        nc.vector.reciprocal(rs8, sd8)
                    pmb = ps.tile([128, 512], F32, tag="lnb", bufs=2,
                                  name=nm("pmb"))[:, :ww]
                    nc.tensor.matmul(pmb, on8b, m8, start=True, stop=True)
                    prb = ps.tile([128, 512], F32, tag="lnb", bufs=2,
                                  name=nm("prb"))[:, :ww]
                    nc.tensor.matmul(prb, on8b, rs8, start=True, stop=True)
                    for k in range(KC):
                        t = tmp.tile([128, 512], F32, tag="t512", bufs=3, name=nm("lt"))[:, :ww]
                        nc.vector.tensor_sub(t, get_x(k, nt), pmb)
                        nc.vector.scalar_tensor_tensor(
                            out=out_t[:, k, w0:w0 + ww], in0=t,
                            scalar=w_col[:, k:k + 1], in1=prb,
                            op0=ALU.mult, op1=ALU.mult)
                        nc.vector.tensor_scalar_add(
                            out_t[:, k, w0:w0 + ww], out_t[:, k, w0:w0 + ww],
                            b_col[:, k:k + 1])

            def wslab(Wdram, col0, ncols, kc0=0, nk=KC):
                t = wp.tile([128, KC, 512], BF16, tag="wslab", name=nm("ws"))[:, :nk, :ncols]
                for k in range(nk):
                    nc.sync.dma_start(
                        out=t[:, k, :],
                        in_=Wdram[128 * (kc0 + k):128 * (kc0 + k + 1),
                                  col0:col0 + ncols])
                return t

            def proj_fm(Wdram, col0, nout, rhs_t, Wtok, copyback):
                for g0 in range(0, nout, 4):
                    gn = min(4, nout - g0)
                    slab = wslab(Wdram, col0 + 128 * g0, 128 * gn)
                    for ml in range(gn):
                        m = g0 + ml
                        for nt in range((Wtok + 511) // 512):
                            w0 = nt * 512
                            ww = min(Wtok, w0 + 512) - w0
                            p = ps.tile([128, 512], F32, tag="mm",
                                        name=nm("pj"))[:, :ww]
                            for k in range(KC):
                                nc.tensor.matmul(
                                    p, slab[:, k, 128 * ml:128 * (ml + 1)],
                                    rhs_t[:, k, w0:w0 + ww],
                                    start=(k == 0), stop=(k == KC - 1))
                            copyback(p, m, w0, ww)

            def attention(kTt, vaugt, q_t, nslots_fn, pad_t, pad_col_fn, out_t,
                          masked):
                for qc in range(2):
                    slots = nslots_fn(qc)
                    for hp in range(8):
                        pys = []
                        for h in (2 * hp, 2 * hp + 1):
                            hb = 64 * (h % 2)
                            py = ps.tile([65, 256], F32, tag="acc", bufs=2,
                                         name=nm("py"))
                            for si, (ck, kind) in enumerate(slots):
                                psc = ps.tile([128, 256], F32, tag="mm",
                                              name=nm("sc"))
                                nc.tensor.matmul(
                                    psc,
                                    kTt[hb:hb + 64, h // 2, 128 * ck:128 * (ck + 1)],
                                    q_t[hb:hb + 64, h // 2, 256 * qc:256 * (qc + 1)],
                                    start=True, stop=True)
                                if masked and kind == 1:
                                    nc.vector.tensor_add(
                                        psc[:, 0:128], psc[:, 0:128], t1m)
                                elif masked and kind == 2:
                                    nc.vector.tensor_add(psc, psc, m1m)
                                ex = exp_pool.tile([128, 256], BF16, tag="ex")
                                col = pad_col_fn(qc, ck)
                                nc.scalar.activation(
                                    ex, psc, AF.Exp,
                                    bias=pad_t[:, col:col + 1], scale=SCALE)
                                nc.tensor.matmul(
                                    py,
                                    vaugt[:, ck, 65 * h:65 * h + 65],
                                    ex,
                                    start=(si == 0), stop=(si == len(slots) - 1))
                            pys.append(py)
                        rsb = tmp.tile([128, 256], F32, tag="rsb", bufs=2)
                        n33 = tmp.tile([33, 256], F32, tag="n33", bufs=2,
                                       name=nm("n33"))
                        nc.vector.memset(n33, 0.0)
                        nc.vector.tensor_copy(n33[0:1, :], pys[0][64:65, :])
                        nc.vector.tensor_copy(n33[32:33, :], pys[1][64:65, :])
                        nc.vector.tensor_scalar_add(n33, n33, 1e-30)
                        rec33 = tmp.tile([33, 256], BF16, tag="rec33", bufs=2,
                                         name=nm("rc"))
                        with nc.allow_low_precision(reason="softmax denom bf16"):
                            nc.vector.reciprocal(rec33, n33)
                        pr = ps.tile([128, 256], F32, tag="mm", name=nm("pr"))
                        nc.tensor.matmul(pr, e2[0:33, :], rec33, start=True, stop=True)
                        nc.vector.tensor_copy(rsb, pr)
                        for j, h in enumerate((2 * hp, 2 * hp + 1)):
                            hb = 64 * (h % 2)
                            nc.vector.tensor_mul(
                                out_t[hb:hb + 64, h // 2, 256 * qc:256 * (qc + 1)],
                                pys[j][0:64, :], rsb[hb:hb + 64, :])

            # ================ phase 1: LN1 + QKV ================
            a1 = act.tile([128, KC, T], BF16, tag="big", bufs=2, name="a1")

            def get_x1_fresh(k, nt):
                t = tmp.tile([128, 512], F32R, tag="xs", bufs=2)
                nc.sync.dma_start(
                    out=t, in_=XT[128 * k:128 * (k + 1), 512 * nt:512 * (nt + 1)])
                return t

            layernorm(get_x1_fresh, T, ln1w, ln1b, a1)

            qT = act.tile([128, KC, TMY], BF16, tag="bb", bufs=2, name="qT")

            def cb_q(p, m, w0, ww):
                nc.vector.tensor_scalar_add(qT[:, m, w0:w0 + ww], p, bqk[:, m:m + 1])

            proj_fm(WQKV, 0, KC, a1, TMY, cb_q)

            kT = act.tile([128, KC, T], BF16, tag="big", bufs=2, name="kT")

            def cb_k(p, m, w0, ww):
                nc.vector.tensor_scalar_add(kT[:, m, w0:w0 + ww], p,
                                            bqk[:, 8 + m:9 + m])

            proj_fm(WQKV, C, KC, a1, T, cb_k)

            vaug = act.tile([128, KC, 16 * 65], BF16, tag="vaug", bufs=1)
            for t8 in range(KC):
                nc.sync.dma_start(
                    out=vaug.rearrange("p c (h q) -> p c h q", q=65)[:, t8, :, 64:65],
                    in_=ONES16.rearrange("p (h q) -> p h q", q=1))
            for g in range(2):
                slab = wslab(WQKV, 2 * C + 512 * g, 512)
                for t8 in range(KC):
                    p = ps.tile([128, 512], F32, tag="mm", name=nm("v"))
                    for k in range(KC):
                        nc.tensor.matmul(p, a1[:, k, 128 * t8:128 * (t8 + 1)],
                                         slab[:, k, :],
                                         start=(k == 0), stop=(k == KC - 1))
                    nc.vector.tensor_add(
                        vaug.rearrange("p c (h q) -> p c h q", q=65)[
                            :, t8, 8 * g:8 * (g + 1), 0:64],
                        p.rearrange("p (h q) -> p h q", q=64),
                        vbbc.rearrange("p (h q) -> p h q", q=64)[
                            :, 8 * g:8 * (g + 1), :])

            # ================ phase 2: self-attention ================
            yT = act.tile([128, KC, TMY], BF16, tag="bb", bufs=2, name="yT")
            attention(kT, vaug, qT, lambda qc: SLOTS[qc], pads,
                      lambda qc, ck: 8 * qc + ck, yT, masked=True)

            # ================ phase 3: out-proj + residual ================
            xr1 = act.tile([128, KC, TMY], F32R, tag="res", bufs=2, name="xr1")

            def cb_z1(p, m, w0, ww):
                z = tmp.tile([128, 512], F32, tag="t512", bufs=3, name=nm("z"))[:, :ww]
                nc.vector.tensor_scalar_add(z, p, bo8[:, m:m + 1])
                zk = tmp.tile([128, 512], F32, tag="t512", bufs=3, name=nm("zk"))[:, :ww]
                nc.vector.tensor_mul(zk, z, keepbc[:, w0:w0 + ww])
                nc.vector.tensor_add(xr1[:, m, w0:w0 + ww], zk, xmy[:, m, w0:w0 + ww])

            proj_fm(WO, 0, KC, yT, TMY, cb_z1)

            # ================ phase 4: cross-attention ================
            a2 = act.tile([128, KC, TMY], BF16, tag="bb", bufs=2, name="a2")
            layernorm(lambda k, nt: xr1[:, k, 512 * nt:512 * (nt + 1)],
                      TMY, lncw, lncb, a2)

            qcT = act.tile([128, KC, TMY], BF16, tag="bb", bufs=2, name="qcT")

            def cb_qc(p, m, w0, ww):
                nc.vector.tensor_scalar_add(qcT[:, m, w0:w0 + ww], p, bq8[:, m:m + 1])

            proj_fm(WQ, 0, KC, a2, TMY, cb_qc)

            kcT = act.tile([128, KC, Tp], BF16, tag="kcT", bufs=1)

            def cb_kc(p, m, w0, ww):
                nc.vector.tensor_scalar_add(kcT[:, m, w0:w0 + ww], p,
                                            bkc8[:, m:m + 1])

            proj_fm(WKV, 0, KC, pkT, Tp, cb_kc)

            vcaug = act.tile([128, 2, 16 * 65], BF16, tag="vcaug", bufs=1)
            for t2 in range(2):
                nc.sync.dma_start(
                    out=vcaug.rearrange("p c (h q) -> p c h q", q=65)[:, t2, :, 64:65],
                    in_=ONES16.rearrange("p (h q) -> p h q", q=1))
            for g in range(2):
                slab = wslab(WKV, C + 512 * g, 512)
                for t2 in range(2):
                    p = ps.tile([128, 512], F32, tag="mm", name=nm("vc"))
                    for k in range(KC):
                        nc.tensor.matmul(p, pkT[:, k, 128 * t2:128 * (t2 + 1)],
                                         slab[:, k, :],
                                         start=(k == 0), stop=(k == KC - 1))
                    nc.vector.tensor_add(
                        vcaug.rearrange("p c (h q) -> p c h q", q=65)[
                            :, t2, 8 * g:8 * (g + 1), 0:64],
                        p.rearrange("p (h q) -> p h q", q=64),
                        vcbbc.rearrange("p (h q) -> p h q", q=64)[
                            :, 8 * g:8 * (g + 1), :])

            ycT = act.tile([128, KC, TMY], BF16, tag="bb", bufs=2, name="ycT")
            attention(kcT, vcaug, qcT, lambda qc: [(0, 0), (1, 0)], padc,
                      lambda qc, ck: ck, ycT, masked=False)

            xr2 = act.tile([128, KC, TMY], F32R, tag="res", bufs=2, name="xr2")

            def cb_z2(p, m, w0, ww):
                z = tmp.tile([128, 512], F32, tag="t512", bufs=3, name=nm("z"))[:, :ww]
                nc.vector.tensor_scalar_add(z, p, bco8[:, m:m + 1])
                zk = tmp.tile([128, 512], F32, tag="t512", bufs=3, name=nm("zk"))[:, :ww]
                nc.vector.tensor_mul(zk, z, keepbc[:, w0:w0 + ww])
                nc.vector.tensor_add(xr2[:, m, w0:w0 + ww], zk, xr1[:, m, w0:w0 + ww])

            proj_fm(WCO, 0, KC, ycT, TMY, cb_z2)

            # ================ phase 5: MLP ================
            a3 = act.tile([128, KC, TMY], BF16, tag="bb", bufs=2, name="a3")
            layernorm(lambda k, nt: xr2[:, k, 512 * nt:512 * (nt + 1)],
                      TMY, ln2w, ln2b, a3)

            hT = act.tile([128, FKC, TMY], BF16, tag="hT", bufs=1)

            def cb_h(p, m, w0, ww):
                nc.scalar.activation(hT[:, m, w0:w0 + ww], p, GELU,
                                     bias=bfc32[:, m:m + 1])

            proj_fm(WFC, 0, FKC, a3, TMY, cb_h)

            outT = act.tile([128, KC, TMY], F32, tag="res", bufs=2, name="outT")
            for g in range(4):
                pouts = [ps.tile([128, 512], F32, tag="acc", bufs=2,
                                 name=nm("fp")) for _ in range(2)]
                for ksb in range(4):
                    slab = wslab(WFP, 256 * g, 256, kc0=8 * ksb, nk=8)
                    for i in range(2):
                        m = 2 * g + i
                        for k in range(8):
                            nc.tensor.matmul(
                                pouts[i], slab[:, k, 128 * i:128 * (i + 1)],
                                hT[:, 8 * ksb + k, :],
                                start=(ksb == 0 and k == 0),
                                stop=(ksb == 3 and k == 7))
                for i in range(2):
                    m = 2 * g + i
                    zf = tmp.tile([128, 512], F32, tag="t512", bufs=3,
                                  name=nm("zf"))
                    nc.vector.tensor_scalar_add(zf, pouts[i], bfp8[:, m:m + 1])
                    nc.vector.tensor_add(outT[:, m, :], zf, xr2[:, m, :])
            for m in range(KC):
                nc.sync.dma_start(out=OUT[128 * m:128 * (m + 1), :],
                                  in_=outT[:, m, :])

    nc.compile()
    return nc


def _host_prepare_general(core, inputs):
    import ml_dtypes
    BF = ml_dtypes.bfloat16
    b, half = core // 2, core % 2
    chunks = [0, 3, 1, 2] if half == 0 else [1, 2, 0, 3]
    idx = np.concatenate([np.arange(256 * c, 256 * (c + 1)) for c in chunks])

    x = np.asarray(inputs["x"], np.float32)
    pad = np.asarray(inputs["padding_mask"], bool)
    pocket = np.asarray(inputs["pocket_context"], np.float32)

    xT = np.ascontiguousarray(x[b][idx].T)
    pkT = np.ascontiguousarray(pocket[b].T).astype(BF)
    pad_loc = pad[b][idx]

    pads = np.zeros((128, 16), np.float32)
    attend = {0: {0, 1} | ({4, 5} if half == 1 else set()),
              1: {0, 1, 2, 3, 4, 5} | ({6, 7} if half == 0 else set())}
    for qc in range(2):
        for ck in range(8):
            col = 8 * qc + ck
            if ck not in attend[qc]:
                pads[:, col] = NEGB
            else:
                pads[:, col] = np.where(pad_loc[128 * ck:128 * (ck + 1)], NEGB, 0.0)
    pmask = np.abs(pocket[b]).sum(-1) == 0
    padc = np.zeros((128, 2), np.float32)
    for ck in range(2):
        padc[:, ck] = np.where(pmask[128 * ck:128 * (ck + 1)], NEGB, 0.0)
    keep = np.where(pad_loc[:TMY], 0.0, 1.0).astype(np.float32)

    t1 = np.where(np.arange(128)[None, :] >= np.arange(128)[:, None], 0.0,
                  NEG).astype(np.float32)
    m1 = np.concatenate([np.full((128, 128), NEG, np.float32), t1], axis=1)
    e2 = np.zeros((64, 128), np.float32)
    e2[0, 0:64] = 1.0
    e2[32, 64:128] = 1.0

    def pp(v):
        return np.ascontiguousarray(np.asarray(v, np.float32).reshape(-1, 128).T)

    def w(name):
        return np.asarray(inputs[name], np.float32).astype(BF)

    bqkv = np.asarray(inputs["bqkv"], np.float32)
    bkv = np.asarray(inputs["bkv"], np.float32)
    return {
        "XT": xT, "XMY": np.ascontiguousarray(xT[:, :TMY]), "PKT": pkT,
        "WQKV": w("Wqkv"), "WO": w("Wo"), "WQ": w("Wq"), "WKV": w("Wkv"),
        "WCO": w("Wco"), "WFC": w("Wfc"), "WFP": w("Wfp"),
        "BQK": pp(bqkv[:2048]), "BO8": pp(inputs["bo"]), "BQ8": pp(inputs["bq"]),
        "BKC8": pp(bkv[:1024]), "BCO8": pp(inputs["bco"]),
        "BFC32": pp(inputs["bfc"]), "BFP8": pp(inputs["bfp"]),
        "LN1W": pp(inputs["ln1_w"]), "LN1B": pp(inputs["ln1_b"]),
        "LNCW": pp(inputs["lnc_w"]), "LNCB": pp(inputs["lnc_b"]),
        "LN2W": pp(inputs["ln2_w"]), "LN2B": pp(inputs["ln2_b"]),
        "T1M": t1, "M1M": m1, "E2": e2.astype(BF),
        "ONL8": np.ones((128, 8), np.float32),
        "ON8B": np.full((8, 128), 0.125, np.float32),
        "EPS8": np.full((8, 1), EPS, np.float32),
        "ONES16": np.ones((128, 16), np.float32).astype(BF),
        "KEEPBC": np.broadcast_to(keep, (128, TMY)).copy(),
        "VBBC": np.broadcast_to(bqkv[2048:], (128, C)).astype(BF).copy(),
        "VCBBC": np.broadcast_to(bkv[1024:], (128, C)).astype(BF).copy(),
        "PADS": pads, "PADC": padc,
    }



# revision 24
# speedup vs baseline: 1.0072x; 1.0072x over previous
"""Trainium2 Bass kernel for a dense transformer block (self-attn + cross-attn + MLP).

Sharding: 8 cores = 4 batches x 2 query-halves. Host permutes tokens per core so
the core's 512 query tokens are local columns 0-511; causal zig-zag chunk
assignment (chunks {0,3} vs {1,2} of 256 tokens) balances attention work, and a
single uniform SPMD program runs on all cores (dummy kv slots masked via
per-core bias data).

Fast path (used when padding_mask is empty, all linear biases are zero, LN
weights are 1/0, and the pocket context has no all-zero rows — which is what
setup_inputs produces): bf16 residual stream (halves x DMA + speeds LN-stats
matmuls), per-chunk DMA splitting (parallel queues), LN statistics pipelined
into the preceding projection/DMA phase, pocket K/V projections run during
the startup x-DMA window, deferred V-projection chunks fill self-attention
qc0 exp-bound gaps, deferred qcT / split WCO projections fill cross-attention
gaps, slot-paired exps ([128,512] per ACT op with pair-uniform mask bias),
softmax denominators via reciprocal_approx_fast, LN rstd via
Exp(-0.5*Ln(var+eps)) so the whole kernel (except GELU) runs from one ACT
table, and bf16 LN normalize (mean/rstd broadcast copied to bf16 once).

General fallback: the original (slower) build, kept verbatim below.
"""
import os
import numpy as np

B, T, Tp, C, H = 4, 1024, 256, 1024, 16
D = C // H           # 64
KC = C // 128        # 8
FF = 4 * C
FKC = FF // 128      # 32
TMY = 512
EPS = 1e-5
NEG = -120000.0      # pre-scale mask; *0.125 = -15000 -> exp == 0
NEGB = -15000.0      # post-scale mask (activation bias)
SCALE = 0.125

_CACHED = {}

# self-attn kv slots per q-chunk: (chunk, kind) kind: 0=full, 1=diag0, 2=diag1
SLOTS = {0: [(4, 0), (5, 0), (0, 1), (1, 2)],
         1: [(0, 0), (1, 0), (4, 0), (5, 0), (6, 0), (7, 0), (2, 1), (3, 2)]}


def _patch_act_tables():
    """Keep Exp/Ln only in the natural_log_exp_and_others set so the
    activation-table placement pass picks the one table containing both —
    the LN tail chain (…Ln→Exp…) and the attention Exps then share a
    single resident table instead of thrashing 1.3us ACT_TABLE_LOADs.
    Removing functions from the other sets is a safe subset-lie: the sets
    keep their names/indices, and every load the pass emits still loads a
    table that really contains the functions used."""
    import concourse.bacc as bacc_mod
    import concourse.hw_specs as hw
    import concourse.mybir as mybir
    if getattr(bacc_mod, "_ant_act_tables_patched", False):
        return
    orig = hw.get_activation_tables
    AF = mybir.ActivationFunctionType

    def patched(arch):
        tabs = {k: set(v) for k, v in orig(arch).items()}
        for name, s in tabs.items():
            if name != "natural_log_exp_and_others":
                s.discard(AF.Exp)
                s.discard(AF.Ln)
        return tabs

    bacc_mod.get_activation_tables = patched
    bacc_mod._ant_act_tables_patched = True


def _build_nc_fast(debug=False):
    import concourse.mybir as mybir
    from concourse import bacc
    from concourse.tile import TileContext

    _patch_act_tables()

    F32 = mybir.dt.float32
    F32R = mybir.dt.float32r
    BF16 = mybir.dt.bfloat16
    AF = mybir.ActivationFunctionType
    ALU = mybir.AluOpType

    nc = bacc.Bacc("TRN2", target_bir_lowering=False, debug=False)

    def din(name, shape, dt):
        return nc.declare_dram_parameter(name, list(shape), dt, isOutput=False)

    # packed inputs: [128, kc*cols] slab-major
    XMY = din("XMY", [128, KC * TMY], BF16)
    XT2 = din("XT2", [128, KC * TMY], BF16)
    PKT = din("PKT", [128, KC * Tp], BF16)
    # weights, slab-packed: slab g = [128, nk*512] contiguous
    WQKVF = din("WQKVF", [128, 6 * KC * 512], BF16)
    WOF = din("WOF", [128, 2 * KC * 512], BF16)
    WQF = din("WQF", [128, 2 * KC * 512], BF16)
    WKVF = din("WKVF", [128, 4 * KC * 512], BF16)
    WCOF = din("WCOF", [128, 2 * KC * 512], BF16)
    WFCF = din("WFCF", [128, 8 * KC * 512], BF16)
    WFPF = din("WFPF", [128, 8 * FKC * 128], BF16)
    TRI01 = din("TRI01", [128, 128], BF16)
    ONL8 = din("ONL8", [128, 8], BF16)
    ON8B = din("ON8B", [8, 128], BF16)
    EPS8 = din("EPS8", [8, 1], F32)
    PADS2 = din("PADS2", [128, 8], F32)
    OUT2 = nc.declare_dram_parameter("OUT2", [128, KC * TMY], BF16, isOutput=True)

    uid = [0]

    def nm(p):
        uid[0] += 1
        return f"{p}_{uid[0]}"

    with TileContext(nc) as tc:
        with tc.tile_pool(name="cst", bufs=1) as cst, \
             tc.tile_pool(name="act", bufs=1) as act, \
             tc.tile_pool(name="wp", bufs=3) as wp, \
             tc.tile_pool(name="tmp", bufs=3) as tmp, \
             tc.tile_pool(name="ex", bufs=6) as exp_pool, \
             tc.tile_pool(name="ps", bufs=4, space="PSUM") as ps:

            # Warm up custom-DVE uop table + gpsimd library BEFORE any tile
            # DMA lands: the table/library staging at kernel start writes
            # SBUF and races with concurrently-landing input DMAs (observed
            # as a +-2.0 constant pattern over early tiles). Every
            # DMA-loaded tile gets a 1-element guard write that depends on
            # the warm-up, ordering its load after the staging completes.
            warm3 = cst.tile([128, 4], F32, tag="warm3")
            nc.vector.memset(warm3, 1.0)

            def guard(ap):
                with nc.allow_low_precision(reason="dma order guard"):
                    nc.vector.tensor_copy(ap, warm3[0:1, 0:1])

            def c_tile(name, dram, shape, dt):
                t = cst.tile(list(shape), dt, tag=name)
                guard(t[0:1, 0:1])
                nc.sync.dma_start(out=t, in_=dram[:, :])
                return t

            tri01 = c_tile("tri01", TRI01, [128, 128], BF16)
            onl8 = c_tile("onl8", ONL8, [128, 8], BF16)
            on8b = c_tile("on8b", ON8B, [8, 128], BF16)
            eps8 = c_tile("eps8", EPS8, [8, 1], F32)
            pads2 = c_tile("pads2", PADS2, [128, 8], F32)

            def wslab(Wf, g, nk=KC, ncols=512, twod=False, tag="wslab",
                      bufs=None):
                t = wp.tile([128, nk, ncols], BF16, tag=tag, name=nm("ws"),
                            bufs=bufs)
                guard(t[0:1, 0, 0:1])
                off = g * nk * ncols
                if twod:
                    for k in range(nk):
                        nc.sync.dma_start(
                            out=t[:, k, :],
                            in_=Wf[:, off + k * ncols:off + (k + 1) * ncols])
                else:
                    nc.sync.dma_start(
                        out=t,
                        in_=Wf[:, off:off + nk * ncols].rearrange(
                            "p (k c) -> p k c", c=ncols))
                return t

            # ---- residual stream x: bf16, chunk-pair DMAs (2KB contiguous
            # rows per partition — fat descriptors — while still letting the
            # LN1 stats start on early chunks). Emission order (= queue
            # order) puts xmy first, then pocket inputs/WKV slabs
            # interleaved with xt2.
            def chunked(dst, src, w, step=2):
                for k in range(0, KC, step):
                    guard(dst[0:1, k, 0:1])
                    nc.sync.dma_start(
                        out=dst[:, k:k + step, :],
                        in_=src[:, k * w:(k + step) * w].rearrange(
                            "p (k c) -> p k c", c=w))

            xmy = act.tile([128, KC, TMY], BF16, tag="res", bufs=2, name="xmy")
            chunked(xmy, XMY, TMY)
            pkT = act.tile([128, KC, Tp], BF16, tag="pkT", bufs=1)
            chunked(pkT, PKT, Tp, step=4)
            kvs = {}
            kvs[0] = wslab(WKVF, 0, tag="kvw", bufs=2)
            xt2 = act.tile([128, KC, TMY], BF16, tag="res", bufs=2, name="xt2")
            chunked(xt2, XT2, TMY)
            kvs[1] = wslab(WKVF, 1, tag="kvw", bufs=2)

            def ln_stats_unit(xs, pm, ps2, k):
                """Accumulate sum(x) / sum(x^2) for one [128,512] chunk.
                pm's matmul goes first (it doesn't wait on the square)."""
                nc.tensor.matmul(pm, onl8, xs,
                                 start=(k == 0), stop=(k == KC - 1))
                sq = tmp.tile([128, 512], BF16, tag="lnsq", bufs=3, name=nm("sq"))
                nc.vector.tensor_mul(sq, xs, xs)
                nc.tensor.matmul(ps2, onl8, sq,
                                 start=(k == 0), stop=(k == KC - 1))

            def ln_tail(pm, ps2):
                """Serial mean/rstd chain; returns bf16 broadcast tiles."""
                m8 = tmp.tile([8, 512], BF16, tag="lnm8", bufs=2, name=nm("m8"))
                nc.scalar.activation(m8, pm, AF.Copy, scale=1.0 / C)
                m2 = tmp.tile([8, 512], F32, tag="lnf", bufs=3, name=nm("m2"))
                nc.scalar.activation(m2, pm, AF.Square, scale=1.0 / C)
                var8 = tmp.tile([8, 512], F32, tag="lnf", bufs=3, name=nm("var8"))
                nc.vector.scalar_tensor_tensor(
                    out=var8, in0=ps2, scalar=1.0 / C, in1=m2,
                    op0=ALU.mult, op1=ALU.subtract)
                l8 = tmp.tile([8, 512], F32, tag="lnf", bufs=3, name=nm("l8"))
                nc.scalar.activation(l8, var8, AF.Ln, bias=eps8[:, :])
                rs8 = tmp.tile([8, 512], BF16, tag="lnm8", bufs=2, name=nm("rs8"))
                nc.scalar.activation(rs8, l8, AF.Exp, scale=-0.5)
                pmb = ps.tile([128, 512], F32, tag="mm", name=nm("pmb"))
                nc.tensor.matmul(pmb, on8b, m8, start=True, stop=True)
                prb = ps.tile([128, 512], F32, tag="mm", name=nm("prb"))
                nc.tensor.matmul(prb, on8b, rs8, start=True, stop=True)
                pmbb = tmp.tile([128, 512], BF16, tag="lnbb", bufs=4,
                                name=nm("pmbb"))
                nc.vector.tensor_copy(pmbb, pmb)
                prbb = tmp.tile([128, 512], BF16, tag="lnbb", bufs=4,
                                name=nm("prbb"))
                nc.vector.tensor_copy(prbb, prb)
                return pmbb, prbb

            def ln_norm(xs_fn, pmbb, prbb, out_t, nt):
                for k in range(KC):
                    t = tmp.tile([128, 512], BF16, tag="lnsq", bufs=3,
                                 name=nm("lt"))
                    nc.vector.tensor_sub(t, xs_fn(k), pmbb)
                    nc.vector.tensor_mul(out_t[:, k, 512 * nt:512 * (nt + 1)],
                                         t, prbb)

            def attention(kTt, vaugt, q_t, slots_fn, pad_col_fn, out_t, masked,
                          fillers=None):
                """out_t [128, KC, TMY] bf16; slot-paired exps. fillers maps
                qc -> list of closures (independent PE work); one is emitted
                per head-pair to keep the PE dense through the exp-bound
                stretches, with leftovers flushed before the next qc."""
                fillers = fillers or {}
                for qc in range(2):
                    flist = fillers.get(qc, [])
                    slots = slots_fn(qc)
                    npair = len(slots) // 2
                    for hp in range(8):
                        if flist:
                            flist.pop(0)()
                        pys = []
                        for h in (2 * hp, 2 * hp + 1):
                            hb = 64 * (h % 2)
                            py = ps.tile([65, 256], F32, tag="acc", bufs=4,
                                         name=nm("py"))
                            for pi in range(npair):
                                psc2 = ps.tile([128, 512], F32, tag="mm",
                                               name=nm("sc"))
                                for j in range(2):
                                    ck, kind = slots[2 * pi + j]
                                    nc.tensor.matmul(
                                        psc2[:, 256 * j:256 * (j + 1)],
                                        kTt[hb:hb + 64, h // 2,
                                            128 * ck:128 * (ck + 1)],
                                        q_t[hb:hb + 64, h // 2,
                                            256 * qc:256 * (qc + 1)],
                                        start=True, stop=True)
                                ex2 = exp_pool.tile([128, 512], BF16, tag="ex")
                                col = pad_col_fn(qc, pi)
                                nc.scalar.activation(
                                    ex2, psc2, AF.Exp,
                                    bias=pads2[:, col:col + 1], scale=SCALE)
                                for j in range(2):
                                    ck, kind = slots[2 * pi + j]
                                    if masked and kind == 1:
                                        nc.vector.tensor_mul(
                                            ex2[:, 256 * j:256 * j + 128],
                                            ex2[:, 256 * j:256 * j + 128], tri01)
                                    elif masked and kind == 2:
                                        nc.vector.memset(
                                            ex2[:, 256 * j:256 * j + 128], 0.0)
                                        nc.vector.tensor_mul(
                                            ex2[:, 256 * j + 128:256 * (j + 1)],
                                            ex2[:, 256 * j + 128:256 * (j + 1)],
                                            tri01)
                                for j in range(2):
                                    ck, _ = slots[2 * pi + j]
                                    nc.tensor.matmul(
                                        py,
                                        vaugt[:, ck, 65 * h:65 * h + 65],
                                        ex2[:, 256 * j:256 * (j + 1)],
                                        start=(pi == 0 and j == 0),
                                        stop=(pi == npair - 1 and j == 1))
                            pys.append(py)
                        for j, h in enumerate((2 * hp, 2 * hp + 1)):
                            hb = 64 * (h % 2)
                            n1 = tmp.tile([1, 256], F32, tag="n1", bufs=2,
                                          name=nm("n1"))
                            nc.vector.tensor_copy(n1, pys[j][64:65, :])
                            rec1 = tmp.tile([1, 256], F32, tag="rec1", bufs=2,
                                            name=nm("rc"))
                            nc.vector.reciprocal_approx_fast(rec1, n1)
                            rsb = tmp.tile([64, 256], F32, tag="rsb", bufs=2,
                                           name=nm("rb"))
                            nc.gpsimd.partition_broadcast(rsb, rec1, channels=64)
                            nc.vector.tensor_mul(
                                out_t[hb:hb + 64, h // 2,
                                      256 * qc:256 * (qc + 1)],
                                pys[j][0:64, :], rsb)
                    while flist:
                        flist.pop(0)()

            # ---- PE clock warm-up: the tensor engine ramps to full clock
            # only after ~3us of continuous work; these throwaway matmuls
            # run while the first x chunks land so the real stats start at
            # full speed instead of the mid p-state.
            wrm = cst.tile([128, 512], BF16, tag="wrm")
            nc.vector.memset(wrm, 0.0)
            for i in range(6):
                pw = ps.tile([128, 512], F32, tag="mm", name=nm("pw"))
                for r in range(4):
                    nc.tensor.matmul(pw, wrm[:, 0:128], wrm,
                                     start=(r == 0), stop=(r == 3))

            # ================ phase 1: LN1 stats (DMA-paced) + pocket K =====
            kcT = act.tile([128, KC, Tp], BF16, tag="kcT", bufs=1)

            def kc_unit(g, ml):
                m = 4 * g + ml
                p = ps.tile([128, 512], F32, tag="mm", name=nm("kc"))[:, :Tp]
                for k in range(KC):
                    nc.tensor.matmul(
                        p, kvs[g][:, k, 128 * ml:128 * (ml + 1)],
                        pkT[:, k, :], start=(k == 0), stop=(k == KC - 1))
                nc.vector.tensor_copy(kcT[:, m, :], p)

            pm0 = ps.tile([8, 512], F32, tag="acc", name="pm0")
            ps20 = ps.tile([8, 512], F32, tag="acc", name="ps20")
            pm1 = ps.tile([8, 512], F32, tag="acc", name="pm1")
            ps21 = ps.tile([8, 512], F32, tag="acc", name="ps21")
            for k in range(KC):
                ln_stats_unit(xmy[:, k, :], pm0, ps20, k)
            for ml in range(4):
                kc_unit(0, ml)
            for k in range(KC):
                ln_stats_unit(xt2[:, k, :], pm1, ps21, k)

            kvs[2] = wslab(WKVF, 2, tag="kvw", bufs=2)
            kvs[3] = wslab(WKVF, 3, tag="kvw", bufs=2)

            # pocket K second half + pocket V surround the LN1 tails so the
            # PE stays busy through the serial mean/rstd chains and the
            # a1 normalize stretch.
            kc_unit(1, 0)
            kc_unit(1, 1)
            pmbb0, prbb0 = ln_tail(pm0, ps20)
            kc_unit(1, 2)
            kc_unit(1, 3)
            pmbb1, prbb1 = ln_tail(pm1, ps21)

            vcaug = act.tile([128, 2, 16 * 65], BF16, tag="vcaug", bufs=1)
            for t2 in range(2):
                nc.vector.memset(
                    vcaug.rearrange("p c (h q) -> p c h q", q=65)[:, t2, :, 64:65],
                    1.0)

            def vc_unit(g, t2, half):
                def f():
                    p = ps.tile([128, 512], F32, tag="mm",
                                name=nm("vc"))[:, 256 * half:256 * (half + 1)]
                    for k in range(KC):
                        nc.tensor.matmul(
                            p, pkT[:, k, 128 * t2:128 * (t2 + 1)],
                            kvs[2 + g][:, k, 256 * half:256 * (half + 1)],
                            start=(k == 0), stop=(k == KC - 1))
                    nc.scalar.copy(
                        vcaug.rearrange("p c (h q) -> p c h q", q=65)[
                            :, t2, 8 * g + 4 * half:8 * g + 4 * (half + 1), 0:64],
                        p.rearrange("p (h q) -> p h q", q=64))
                return f

            # all vc halves fill self-attention qc1's exp-bound gaps (their
            # WKV slabs land too late to run before the QKV projections)
            vc_deferred = [vc_unit(g, t2, half)
                           for g in range(2) for t2 in range(2)
                           for half in range(2)]

            a1 = act.tile([128, KC, T], BF16, tag="big", bufs=2, name="a1")
            ln_norm(lambda k: xmy[:, k, :], pmbb0, prbb0, a1, 0)
            ln_norm(lambda k: xt2[:, k, :], pmbb1, prbb1, a1, 1)

            # ================ phase 2: QKV projections ================
            qT = act.tile([128, KC, TMY], BF16, tag="bb", bufs=2, name="qT")
            for g in range(2):
                slab = wslab(WQKVF, g)
                for ml in range(4):
                    m = 4 * g + ml
                    p = ps.tile([128, 512], F32, tag="mm", name=nm("q"))
                    for k in range(KC):
                        nc.tensor.matmul(p, slab[:, k, 128 * ml:128 * (ml + 1)],
                                         a1[:, k, 0:TMY],
                                         start=(k == 0), stop=(k == KC - 1))
                    nc.scalar.copy(qT[:, m, :], p)

            kT = act.tile([128, KC, T], BF16, tag="big", bufs=2, name="kT")
            for g in range(2):
                slab = wslab(WQKVF, 2 + g)
                for ml in range(4):
                    m = 4 * g + ml
                    for nt in range(2):
                        p = ps.tile([128, 512], F32, tag="mm", name=nm("k"))
                        for k in range(KC):
                            nc.tensor.matmul(
                                p, slab[:, k, 128 * ml:128 * (ml + 1)],
                                a1[:, k, 512 * nt:512 * (nt + 1)],
                                start=(k == 0), stop=(k == KC - 1))
                        nc.vector.tensor_copy(kT[:, m, 512 * nt:512 * (nt + 1)], p)

            vaug = act.tile([128, KC, 16 * 65], BF16, tag="vaug", bufs=1)
            for t8 in range(KC):
                nc.vector.memset(
                    vaug.rearrange("p c (h q) -> p c h q", q=65)[:, t8, :, 64:65],
                    1.0)
            vslab = [wslab(WQKVF, 4), wslab(WQKVF, 5)]

            def v_unit(g, t8):
                def f():
                    p = ps.tile([128, 512], F32, tag="mm", name=nm("v"))
                    for k in range(KC):
                        nc.tensor.matmul(p, a1[:, k, 128 * t8:128 * (t8 + 1)],
                                         vslab[g][:, k, :],
                                         start=(k == 0), stop=(k == KC - 1))
                    nc.scalar.copy(
                        vaug.rearrange("p c (h q) -> p c h q", q=65)[
                            :, t8, 8 * g:8 * (g + 1), 0:64],
                        p.rearrange("p (h q) -> p h q", q=64))
                return f

            # V for the kv chunks qc0 touches; chunks {2,3,6,7} (only needed
            # from qc1) are deferred into qc0's exp-bound stretches.
            for t8 in (0, 1, 4, 5):
                for g in range(2):
                    v_unit(g, t8)()
            v_deferred = [v_unit(g, t8) for t8 in (2, 3, 6, 7) for g in range(2)]

            # ================ phase 3: self-attention ================
            yT = act.tile([128, KC, TMY], BF16, tag="bb", bufs=2, name="yT")
            attention(kT, vaug, qT, lambda qc: SLOTS[qc],
                      lambda qc, pi: (0 if qc == 0 else 2) + pi, yT, masked=True,
                      fillers={0: v_deferred, 1: vc_deferred})

            # ================ phase 4: out-proj + residual + LNc stats ======
            xr1 = act.tile([128, KC, TMY], BF16, tag="res", bufs=2, name="xr1")
            pmc = ps.tile([8, 512], F32, tag="acc", name="pmc")
            ps2c = ps.tile([8, 512], F32, tag="acc", name="ps2c")
            for g in range(2):
                slab = wslab(WOF, g)
                for ml in range(4):
                    m = 4 * g + ml
                    p = ps.tile([128, 512], F32, tag="mm", name=nm("o"))
                    for k in range(KC):
                        nc.tensor.matmul(p, slab[:, k, 128 * ml:128 * (ml + 1)],
                                         yT[:, k, :],
                                         start=(k == 0), stop=(k == KC - 1))
                    nc.vector.tensor_add(xr1[:, m, :], p, xmy[:, m, :])
                    ln_stats_unit(xr1[:, m, :], pmc, ps2c, m)

            # ================ phase 5: cross-attention ================
            pmbbc, prbbc = ln_tail(pmc, ps2c)
            a2 = act.tile([128, KC, TMY], BF16, tag="bb", bufs=2, name="a2")
            ln_norm(lambda k: xr1[:, k, :], pmbbc, prbbc, a2, 0)

            qcT = act.tile([128, KC, TMY], BF16, tag="bb", bufs=2, name="qcT")
            qslab = [wslab(WQF, 0), wslab(WQF, 1)]

            def qc_unit(g, ml):
                def f():
                    m = 4 * g + ml
                    p = ps.tile([128, 512], F32, tag="mm", name=nm("cq"))
                    for k in range(KC):
                        nc.tensor.matmul(p, qslab[g][:, k, 128 * ml:128 * (ml + 1)],
                                         a2[:, k, :],
                                         start=(k == 0), stop=(k == KC - 1))
                    nc.scalar.copy(qcT[:, m, :], p)
                return f

            qc_unit(0, 0)()
            q_deferred = [qc_unit(m // 4, m % 4) for m in range(1, KC)]

            # WCO projection of qc0's columns runs as qc1 fillers
            ycT = act.tile([128, KC, TMY], BF16, tag="bb", bufs=2, name="ycT")
            xr2 = act.tile([128, KC, TMY], BF16, tag="res", bufs=2, name="xr2")
            coslab = [wslab(WCOF, 0), wslab(WCOF, 1)]

            def wco_half0(m):
                def f():
                    p = ps.tile([128, 512], F32, tag="mm", name=nm("co"))[:, :256]
                    for k in range(KC):
                        nc.tensor.matmul(
                            p, coslab[m // 4][:, k, 128 * (m % 4):128 * (m % 4 + 1)],
                            ycT[:, k, 0:256],
                            start=(k == 0), stop=(k == KC - 1))
                    nc.vector.tensor_add(xr2[:, m, 0:256], p, xr1[:, m, 0:256])
                return f

            wco_units = [wco_half0(m) for m in range(KC)]
            attention(kcT, vcaug, qcT, lambda qc: [(0, 0), (1, 0)],
                      lambda qc, pi: 6, ycT, masked=False,
                      fillers={0: q_deferred, 1: wco_units[:6]})
            # last two qc0-column WCO units cover the ycT drain after the
            # final cross head-pair
            for f in wco_units[6:]:
                f()

            # ================ phase 6: WCO half 1 + residual + LN2 stats ====
            pm2t = ps.tile([8, 512], F32, tag="acc", name="pm2t")
            ps22t = ps.tile([8, 512], F32, tag="acc", name="ps22t")
            for m in range(KC):
                p = ps.tile([128, 512], F32, tag="mm", name=nm("co2"))[:, :256]
                for k in range(KC):
                    nc.tensor.matmul(
                        p, coslab[m // 4][:, k, 128 * (m % 4):128 * (m % 4 + 1)],
                        ycT[:, k, 256:512],
                        start=(k == 0), stop=(k == KC - 1))
                nc.vector.tensor_add(xr2[:, m, 256:512], p, xr1[:, m, 256:512])
                ln_stats_unit(xr2[:, m, :], pm2t, ps22t, m)

            # ================ phase 7: MLP ================
            pmbb2, prbb2 = ln_tail(pm2t, ps22t)
            a3 = act.tile([128, KC, TMY], BF16, tag="bb", bufs=2, name="a3")
            ln_norm(lambda k: xr2[:, k, :], pmbb2, prbb2, a3, 0)

            hT = act.tile([128, FKC, TMY], BF16, tag="hT", bufs=1)
            for g in range(8):
                slab = wslab(WFCF, g)
                for ml in range(4):
                    m = 4 * g + ml
                    p = ps.tile([128, 512], F32, tag="mm", name=nm("fc"))
                    for k in range(KC):
                        nc.tensor.matmul(p, slab[:, k, 128 * ml:128 * (ml + 1)],
                                         a3[:, k, :],
                                         start=(k == 0), stop=(k == KC - 1))
                    nc.scalar.activation(hT[:, m, :], p, AF.Gelu)

            outT = act.tile([128, KC, TMY], BF16, tag="res", bufs=2, name="outT")
            for m in range(KC):
                slab = wslab(WFPF, m, nk=FKC, ncols=128)
                pout = ps.tile([128, 512], F32, tag="acc", bufs=4, name=nm("fp"))
                for k in range(FKC):
                    nc.tensor.matmul(pout, slab[:, k, :], hT[:, k, :],
                                     start=(k == 0), stop=(k == FKC - 1))
                nc.vector.tensor_add(outT[:, m, :], pout, xr2[:, m, :])
                nc.sync.dma_start(out=OUT2[:, m * TMY:(m + 1) * TMY],
                                  in_=outT[:, m, :])

    nc.compile()
    return nc


def _slabify(W, slab_cols=512):
    """[K, N] -> [128, (N//slab_cols) * (K//128) * slab_cols] slab-major."""
    K, N = W.shape
    kc = K // 128
    G = N // slab_cols
    Wr = W.reshape(kc, 128, G, slab_cols).transpose(1, 2, 0, 3)
    return np.ascontiguousarray(Wr.reshape(128, G * kc * slab_cols))


def _pack_tok(xT, lo, hi):
    """[C, T] fp32 -> [128, KC*(hi-lo)] k-major."""
    w = hi - lo
    return np.ascontiguousarray(
        xT.reshape(KC, 128, -1)[:, :, lo:hi].transpose(1, 0, 2).reshape(128, KC * w))


def _host_prepare_fast(inputs):
    import ml_dtypes
    BF = ml_dtypes.bfloat16

    shared = {
        "WQKVF": _slabify(np.asarray(inputs["Wqkv"], np.float32)).astype(BF),
        "WOF": _slabify(np.asarray(inputs["Wo"], np.float32)).astype(BF),
        "WQF": _slabify(np.asarray(inputs["Wq"], np.float32)).astype(BF),
        "WKVF": _slabify(np.asarray(inputs["Wkv"], np.float32)).astype(BF),
        "WCOF": _slabify(np.asarray(inputs["Wco"], np.float32)).astype(BF),
        "WFCF": _slabify(np.asarray(inputs["Wfc"], np.float32)).astype(BF),
        "WFPF": _slabify(np.asarray(inputs["Wfp"], np.float32), 128).astype(BF),
        "TRI01": np.where(np.arange(128)[None, :] >= np.arange(128)[:, None],
                          1.0, 0.0).astype(BF),
        "ONL8": np.ones((128, 8), np.float32).astype(BF),
        "ON8B": np.full((8, 128), 0.125, np.float32).astype(BF),
        "EPS8": np.full((8, 1), EPS, np.float32),
    }

    x = np.asarray(inputs["x"], np.float32)
    pocket = np.asarray(inputs["pocket_context"], np.float32)

    attend = {}
    for half in range(2):
        attend[half] = {0: {0, 1} | ({4, 5} if half == 1 else set()),
                        1: {0, 1, 2, 3, 4, 5} | ({6, 7} if half == 0 else set())}

    maps = []
    for core in range(8):
        b, half = core // 2, core % 2
        chunks = [0, 3, 1, 2] if half == 0 else [1, 2, 0, 3]
        idx = np.concatenate([np.arange(256 * c, 256 * (c + 1)) for c in chunks])
        xT = np.ascontiguousarray(x[b][idx].T)
        pkT = np.ascontiguousarray(pocket[b].T)

        pads2 = np.zeros((128, 8), np.float32)
        for qc in range(2):
            slots = SLOTS[qc]
            for pi in range(len(slots) // 2
                            ):
                ckA, ckB = slots[2 * pi][0], slots[2 * pi + 1][0]
                att_a = ckA in attend[half][qc]
                att_b = ckB in attend[half][qc]
                assert att_a == att_b, "pair attend status must be uniform"
                col = (0 if qc == 0 else 2) + pi
                pads2[:, col] = 0.0 if att_a else NEGB
        # col 6 = cross pair: always attended (no pocket padding on fast path)

        maps.append(dict(shared,
                         XMY=_pack_tok(xT, 0, TMY).astype(BF),
                         XT2=_pack_tok(xT, TMY, T).astype(BF),
                         PKT=_pack_tok(pkT, 0, Tp).astype(BF),
                         PADS2=pads2))
    return maps


def _is_fast(inputs):
    z = lambda name: not np.any(np.asarray(inputs[name]))
    ones = lambda name: np.all(np.asarray(inputs[name]) == 1.0)
    pocket = np.asarray(inputs["pocket_context"], np.float32)
    return (not np.any(np.asarray(inputs["padding_mask"]))
            and np.all(np.abs(pocket).sum(-1) != 0)
            and all(z(n) for n in ("bqkv", "bo", "bq", "bkv", "bco", "bfc", "bfp"))
            and all(z(n) for n in ("ln1_b", "lnc_b", "ln2_b"))
            and all(ones(n) for n in ("ln1_w", "lnc_w", "ln2_w")))


LAST_RESULTS = None


def kernel(**inputs):
    global LAST_RESULTS
    from concourse.bass_utils import run_bass_kernel_spmd

    trace = bool(int(os.environ.get("KERNEL_TRACE", "0")))
    if _is_fast(inputs):
        if "nc_fast" not in _CACHED:
            _CACHED["nc_fast"] = _build_nc_fast()
        nc = _CACHED["nc_fast"]
        in_maps = _host_prepare_fast(inputs)
        res = run_bass_kernel_spmd(nc, in_maps, list(range(8)), trace=trace)
        LAST_RESULTS = res
        out = np.zeros((B, T, C), np.float32)
        for core in range(8):
            b, half = core // 2, core % 2
            chunks = [0, 3] if half == 0 else [1, 2]
            o2 = np.asarray(res.results[core]["OUT2"], np.float32)
            o = o2.reshape(128, KC, TMY).transpose(1, 0, 2).reshape(C, TMY)
            for j, c in enumerate(chunks):
                out[b, 256 * c:256 * (c + 1), :] = o[:, 256 * j:256 * (j + 1)].T
        return out

    # -------- general fallback (original baseline path) --------
    if "nc_gen" not in _CACHED:
        _CACHED["nc_gen"] = _build_nc_general()
    nc = _CACHED["nc_gen"]
    in_maps = [_host_prepare_general(core, inputs) for core in range(8)]
    res = run_bass_kernel_spmd(nc, in_maps, list(range(8)), trace=trace)
    LAST_RESULTS = res
    out = np.zeros((B, T, C), np.float32)
    for core in range(8):
        b, half = core // 2, core % 2
        chunks = [0, 3] if half == 0 else [1, 2]
        o = np.asarray(res.results[core]["OUT"])
        for j, c in enumerate(chunks):
            out[b, 256 * c:256 * (c + 1), :] = o[:, 256 * j:256 * (j + 1)].T
    return out


def _build_nc_general(sim_mode=False):
    import concourse.mybir as mybir
    from concourse import bacc
    from concourse.tile import TileContext

    F32 = mybir.dt.float32
    F32R = mybir.dt.float32r
    BF16 = mybir.dt.bfloat16
    AF = mybir.ActivationFunctionType
    ALU = mybir.AluOpType
    GELU = AF.Tanh if sim_mode else AF.Gelu

    nc = bacc.Bacc("TRN2", target_bir_lowering=False, debug=False)

    def din(name, shape, dt):
        return nc.declare_dram_parameter(name, list(shape), dt, isOutput=False)

    XT = din("XT", [C, T], F32R)
    XMY = din("XMY", [C, TMY], F32R)
    PKT = din("PKT", [C, Tp], BF16)
    WQKV = din("WQKV", [C, 3 * C], BF16)
    WO = din("WO", [C, C], BF16)
    WQ = din("WQ", [C, C], BF16)
    WKV = din("WKV", [C, 2 * C], BF16)
    WCO = din("WCO", [C, C], BF16)
    WFC = din("WFC", [C, FF], BF16)
    WFP = din("WFP", [FF, C], BF16)
    BQK = din("BQK", [128, 16], F32)
    BO8 = din("BO8", [128, KC], F32)
    BQ8 = din("BQ8", [128, KC], F32)
    BKC8 = din("BKC8", [128, KC], F32)
    BCO8 = din("BCO8", [128, KC], F32)
    BFC32 = din("BFC32", [128, FKC], F32)
    BFP8 = din("BFP8", [128, KC], F32)
    LN1W = din("LN1W", [128, KC], F32)
    LN1B = din("LN1B", [128, KC], F32)
    LNCW = din("LNCW", [128, KC], F32)
    LNCB = din("LNCB", [128, KC], F32)
    LN2W = din("LN2W", [128, KC], F32)
    LN2B = din("LN2B", [128, KC], F32)
    T1M = din("T1M", [128, 128], F32)
    M1M = din("M1M", [128, 256], F32)
    E2 = din("E2", [64, 128], BF16)
    ONL8 = din("ONL8", [128, 8], F32R)
    ON8B = din("ON8B", [8, 128], F32R)
    EPS8 = din("EPS8", [8, 1], F32)
    ONES16 = din("ONES16", [128, 16], BF16)
    KEEPBC = din("KEEPBC", [128, TMY], F32)
    VBBC = din("VBBC", [128, C], BF16)
    VCBBC = din("VCBBC", [128, C], BF16)
    PADS = din("PADS", [128, 16], F32)
    PADC = din("PADC", [128, 2], F32)
    OUT = nc.declare_dram_parameter("OUT", [C, TMY], F32, isOutput=True)

    uid = [0]

    def nm(p):
        uid[0] += 1
        return f"{p}_{uid[0]}"

    with TileContext(nc) as tc:
        with tc.tile_pool(name="cst", bufs=1) as cst, \
             tc.tile_pool(name="act", bufs=1) as act, \
             tc.tile_pool(name="wp", bufs=2) as wp, \
             tc.tile_pool(name="tmp", bufs=3) as tmp, \
             tc.tile_pool(name="ex", bufs=6) as exp_pool, \
             tc.tile_pool(name="ps", bufs=4, space="PSUM") as ps:

            def c_tile(name, dram, shape, dt):
                t = cst.tile(list(shape), dt, tag=name)
                nc.sync.dma_start(out=t, in_=dram[:, :])
                return t

            t1m = c_tile("t1m", T1M, [128, 128], F32)
            m1m = c_tile("m1m", M1M, [128, 256], F32)
            e2 = c_tile("e2", E2, [64, 128], BF16)
            onl8 = c_tile("onl8", ONL8, [128, 8], F32R)
            on8b = c_tile("on8b", ON8B, [8, 128], F32R)
            eps8 = c_tile("eps8", EPS8, [8, 1], F32)
            ones16 = c_tile("ones16", ONES16, [128, 16], BF16)
            keepbc = c_tile("keepbc", KEEPBC, [128, TMY], F32)
            vbbc = c_tile("vbbc", VBBC, [128, C], BF16)
            vcbbc = c_tile("vcbbc", VCBBC, [128, C], BF16)
            pads = c_tile("pads", PADS, [128, 16], F32)
            padc = c_tile("padc", PADC, [128, 2], F32)
            bqk = c_tile("bqk", BQK, [128, 16], F32)
            bo8 = c_tile("bo8", BO8, [128, KC], F32)
            bq8 = c_tile("bq8", BQ8, [128, KC], F32)
            bkc8 = c_tile("bkc8", BKC8, [128, KC], F32)
            bco8 = c_tile("bco8", BCO8, [128, KC], F32)
            bfc32 = c_tile("bfc32", BFC32, [128, FKC], F32)
            bfp8 = c_tile("bfp8", BFP8, [128, KC], F32)
            ln1w = c_tile("ln1w", LN1W, [128, KC], F32)
            ln1b = c_tile("ln1b", LN1B, [128, KC], F32)
            lncw = c_tile("lncw", LNCW, [128, KC], F32)
            lncb = c_tile("lncb", LNCB, [128, KC], F32)
            ln2w = c_tile("ln2w", LN2W, [128, KC], F32)
            ln2b = c_tile("ln2b", LN2B, [128, KC], F32)

            xmy = act.tile([128, KC, TMY], F32R, tag="res", bufs=2, name="xmy")
            for k in range(KC):
                nc.sync.dma_start(out=xmy[:, k, :], in_=XMY[128 * k:128 * (k + 1), :])
            pkT = act.tile([128, KC, Tp], BF16, tag="pkT", bufs=1)
            for k in range(KC):
                nc.sync.dma_start(out=pkT[:, k, :], in_=PKT[128 * k:128 * (k + 1), :])

            def layernorm(get_x, W, w_col, b_col, out_t):
                ntile = (W + 511) // 512
                for nt in range(ntile):
                    w0 = nt * 512
                    ww = min(W, w0 + 512) - w0
                    xs = [get_x(k, nt) for k in range(KC)]
                    pm = ps.tile([8, 512], F32, tag="mm", name=nm("pm"))[:, :ww]
                    ps2 = ps.tile([8, 512], F32, tag="mm", name=nm("ps2"))[:, :ww]
                    for k in range(KC):
                        sq = tmp.tile([128, 512], F32R, tag="lnsq", bufs=2, name=nm("sq"))[:, :ww]
                        nc.scalar.activation(sq, xs[k], AF.Square)
                        nc.tensor.matmul(pm, onl8, xs[k],
                                         start=(k == 0), stop=(k == KC - 1))
                        nc.tensor.matmul(ps2, onl8, sq,
                                         start=(k == 0), stop=(k == KC - 1))
                    m8 = tmp.tile([8, 512], F32R, tag="lnm8", bufs=2, name=nm("m8"))[:, :ww]
                    nc.scalar.activation(m8, pm, AF.Copy, scale=1.0 / C)
                    m2 = tmp.tile([8, 512], F32, tag="lnf", bufs=3, name=nm("m2"))[:, :ww]
                    nc.scalar.activation(m2, pm, AF.Square, scale=1.0 / C)
                    var8 = tmp.tile([8, 512], F32, tag="lnf", bufs=3, name=nm("var8"))[:, :ww]
                    nc.vector.scalar_tensor_tensor(
                        out=var8, in0=ps2, scalar=1.0 / C, in1=m2,
                        op0=ALU.mult, op1=ALU.subtract)
                    sd8 = tmp.tile([8, 512], F32, tag="lnf", bufs=3, name=nm("sd8"))[:, :ww]
                    nc.scalar.activation(sd8, var8, AF.Sqrt, bias=eps8[:, :])
                    rs8 = tmp.tile([8, 512], F32R, tag="lnm8", bufs=2, name=nm("rs8"))[:, :ww]
                    with nc.allow_low_precision(reason="ln rstd f32r"):
                        nc.vector.reciprocal(rs8, sd8)
                    pmb = ps.tile([128, 512], F32, tag="lnb", bufs=2,
                                  name=nm("pmb"))[:, :ww]
                    nc.tensor.matmul(pmb, on8b, m8, start=True, stop=True)
                    prb = ps.tile([128, 512], F32, tag="lnb", bufs=2,
                                  name=nm("prb"))[:, :ww]
                    nc.tensor.matmul(prb, on8b, rs8, start=True, stop=True)
                    for k in range(KC):
                        t = tmp.tile([128, 512], F32, tag="t512", bufs=3, name=nm("lt"))[:, :ww]
                        nc.vector.tensor_sub(t, get_x(k, nt), pmb)
                        nc.vector.scalar_tensor_tensor(
                            out=out_t[:, k, w0:w0 + ww], in0=t,
                            scalar=w_col[:, k:k + 1], in1=prb,
                            op0=ALU.mult, op1=ALU.mult)
                        nc.vector.tensor_scalar_add(
                            out_t[:, k, w0:w0 + ww], out_t[:, k, w0:w0 + ww],
                            b_col[:, k:k + 1])

            def wslab(Wdram, col0, ncols, kc0=0, nk=KC):
                t = wp.tile([128, KC, 512], BF16, tag="wslab", name=nm("ws"))[:, :nk, :ncols]
                for k in range(nk):
                    nc.sync.dma_start(
                        out=t[:, k, :],
                        in_=Wdram[128 * (kc0 + k):128 * (kc0 + k + 1),
                                  col0:col0 + ncols])
                return t

            def proj_fm(Wdram, col0, nout, rhs_t, Wtok, copyback):
                for g0 in range(0, nout, 4):
                    gn = min(4, nout - g0)
                    slab = wslab(Wdram, col0 + 128 * g0, 128 * gn)
                    for ml in range(gn):
                        m = g0 + ml
                        for nt in range((Wtok + 511) // 512):
                            w0 = nt * 512
                            ww = min(Wtok, w0 + 512) - w0
                            p = ps.tile([128, 512], F32, tag="mm",
                                        name=nm("pj"))[:, :ww]
                            for k in range(KC):
                                nc.tensor.matmul(
                                    p, slab[:, k, 128 * ml:128 * (ml + 1)],
                                    rhs_t[:, k, w0:w0 + ww],
                                    start=(k == 0), stop=(k == KC - 1))
                            copyback(p, m, w0, ww)

            def attention(kTt, vaugt, q_t, nslots_fn, pad_t, pad_col_fn, out_t,
                          masked):
                for qc in range(2):
                    slots = nslots_fn(qc)
                    for hp in range(8):
                        pys = []
                        for h in (2 * hp, 2 * hp + 1):
                            hb = 64 * (h % 2)
                            py = ps.tile([65, 256], F32, tag="acc", bufs=2,
                                         name=nm("py"))
                            for si, (ck, kind) in enumerate(slots):
                                psc = ps.tile([128, 256], F32, tag="mm",
                                              name=nm("sc"))
                                nc.tensor.matmul(
                                    psc,
                                    kTt[hb:hb + 64, h // 2, 128 * ck:128 * (ck + 1)],
                                    q_t[hb:hb + 64, h // 2, 256 * qc:256 * (qc + 1)],
                                    start=True, stop=True)
                                if masked and kind == 1:
                                    nc.vector.tensor_add(
                                        psc[:, 0:128], psc[:, 0:128], t1m)
                                elif masked and kind == 2:
                                    nc.vector.tensor_add(psc, psc, m1m)
                                ex = exp_pool.tile([128, 256], BF16, tag="ex")
                                col = pad_col_fn(qc, ck)
                                nc.scalar.activation(
                                    ex, psc, AF.Exp,
                                    bias=pad_t[:, col:col + 1], scale=SCALE)
                                nc.tensor.matmul(
                                    py,
                                    vaugt[:, ck, 65 * h:65 * h + 65],
                                    ex,
                                    start=(si == 0), stop=(si == len(slots) - 1))
                            pys.append(py)
                        rsb = tmp.tile([128, 256], F32, tag="rsb", bufs=2)
                        n33 = tmp.tile([33, 256], F32, tag="n33", bufs=2,
                                       name=nm("n33"))
                        nc.vector.memset(n33, 0.0)
                        nc.vector.tensor_copy(n33[0:1, :], pys[0][64:65, :])
                        nc.vector.tensor_copy(n33[32:33, :], pys[1][64:65, :])
                        nc.vector.tensor_scalar_add(n33, n33, 1e-30)
                        rec33 = tmp.tile([33, 256], BF16, tag="rec33", bufs=2,
                                         name=nm("rc"))
                        with nc.allow_low_precision(reason="softmax denom bf16"):
                            nc.vector.reciprocal(rec33, n33)
                        pr = ps.tile([128, 256], F32, tag="mm", name=nm("pr"))
                        nc.tensor.matmul(pr, e2[0:33, :], rec33, start=True, stop=True)
                        nc.vector.tensor_copy(rsb, pr)
                        for j, h in enumerate((2 * hp, 2 * hp + 1)):
                            hb = 64 * (h % 2)
                            nc.vector.tensor_mul(
                                out_t[hb:hb + 64, h // 2, 256 * qc:256 * (qc + 1)],
                                pys[j][0:64, :], rsb[hb:hb + 64, :])

            # ================ phase 1: LN1 + QKV ================
            a1 = act.tile([128, KC, T], BF16, tag="big", bufs=2, name="a1")

            def get_x1_fresh(k, nt):
                t = tmp.tile([128, 512], F32R, tag="xs", bufs=2)
                nc.sync.dma_start(
                    out=t, in_=XT[128 * k:128 * (k + 1), 512 * nt:512 * (nt + 1)])
                return t

            layernorm(get_x1_fresh, T, ln1w, ln1b, a1)

            qT = act.tile([128, KC, TMY], BF16, tag="bb", bufs=2, name="qT")

            def cb_q(p, m, w0, ww):
                nc.vector.tensor_scalar_add(qT[:, m, w0:w0 + ww], p, bqk[:, m:m + 1])

            proj_fm(WQKV, 0, KC, a1, TMY, cb_q)

            kT = act.tile([128, KC, T], BF16, tag="big", bufs=2, name="kT")

            def cb_k(p, m, w0, ww):
                nc.vector.tensor_scalar_add(kT[:, m, w0:w0 + ww], p,
                                            bqk[:, 8 + m:9 + m])

            proj_fm(WQKV, C, KC, a1, T, cb_k)

            vaug = act.tile([128, KC, 16 * 65], BF16, tag="vaug", bufs=1)
            for t8 in range(KC):
                nc.sync.dma_start(
                    out=vaug.rearrange("p c (h q) -> p c h q", q=65)[:, t8, :, 64:65],
                    in_=ONES16.rearrange("p (h q) -> p h q", q=1))
            for g in range(2):
                slab = wslab(WQKV, 2 * C + 512 * g, 512)
                for t8 in range(KC):
                    p = ps.tile([128, 512], F32, tag="mm", name=nm("v"))
                    for k in range(KC):
                        nc.tensor.matmul(p, a1[:, k, 128 * t8:128 * (t8 + 1)],
                                         slab[:, k, :],
                                         start=(k == 0), stop=(k == KC - 1))
                    nc.vector.tensor_add(
                        vaug.rearrange("p c (h q) -> p c h q", q=65)[
                            :, t8, 8 * g:8 * (g + 1), 0:64],
                        p.rearrange("p (h q) -> p h q", q=64),
                        vbbc.rearrange("p (h q) -> p h q", q=64)[
                            :, 8 * g:8 * (g + 1), :])

            # ================ phase 2: self-attention ================
            yT = act.tile([128, KC, TMY], BF16, tag="bb", bufs=2, name="yT")
            attention(kT, vaug, qT, lambda qc: SLOTS[qc], pads,
                      lambda qc, ck: 8 * qc + ck, yT, masked=True)

            # ================ phase 3: out-proj + residual ================
            xr1 = act.tile([128, KC, TMY], F32R, tag="res", bufs=2, name="xr1")

            def cb_z1(p, m, w0, ww):
                z = tmp.tile([128, 512], F32, tag="t512", bufs=3, name=nm("z"))[:, :ww]
                nc.vector.tensor_scalar_add(z, p, bo8[:, m:m + 1])
                zk = tmp.tile([128, 512], F32, tag="t512", bufs=3, name=nm("zk"))[:, :ww]
                nc.vector.tensor_mul(zk, z, keepbc[:, w0:w0 + ww])
                nc.vector.tensor_add(xr1[:, m, w0:w0 + ww], zk, xmy[:, m, w0:w0 + ww])

            proj_fm(WO, 0, KC, yT, TMY, cb_z1)

            # ================ phase 4: cross-attention ================
            a2 = act.tile([128, KC, TMY], BF16, tag="bb", bufs=2, name="a2")
            layernorm(lambda k, nt: xr1[:, k, 512 * nt:512 * (nt + 1)],
                      TMY, lncw, lncb, a2)

            qcT = act.tile([128, KC, TMY], BF16, tag="bb", bufs=2, name="qcT")

            def cb_qc(p, m, w0, ww):
                nc.vector.tensor_scalar_add(qcT[:, m, w0:w0 + ww], p, bq8[:, m:m + 1])

            proj_fm(WQ, 0, KC, a2, TMY, cb_qc)

            kcT = act.tile([128, KC, Tp], BF16, tag="kcT", bufs=1)

            def cb_kc(p, m, w0, ww):
                nc.vector.tensor_scalar_add(kcT[:, m, w0:w0 + ww], p,
                                            bkc8[:, m:m + 1])

            proj_fm(WKV, 0, KC, pkT, Tp, cb_kc)

            vcaug = act.tile([128, 2, 16 * 65], BF16, tag="vcaug", bufs=1)
            for t2 in range(2):
                nc.sync.dma_start(
                    out=vcaug.rearrange("p c (h q) -> p c h q", q=65)[:, t2, :, 64:65],
                    in_=ONES16.rearrange("p (h q) -> p h q", q=1))
            for g in range(2):
                slab = wslab(WKV, C + 512 * g, 512)
                for t2 in range(2):
                    p = ps.tile([128, 512], F32, tag="mm", name=nm("vc"))
                    for k in range(KC):
                        nc.tensor.matmul(p, pkT[:, k, 128 * t2:128 * (t2 + 1)],
                                         slab[:, k, :],
                                         start=(k == 0), stop=(k == KC - 1))
                    nc.vector.tensor_add(
                        vcaug.rearrange("p c (h q) -> p c h q", q=65)[
                            :, t2, 8 * g:8 * (g + 1), 0:64],
                        p.rearrange("p (h q) -> p h q", q=64),
                        vcbbc.rearrange("p (h q) -> p h q", q=64)[
                            :, 8 * g:8 * (g + 1), :])

            ycT = act.tile([128, KC, TMY], BF16, tag="bb", bufs=2, name="ycT")
            attention(kcT, vcaug, qcT, lambda qc: [(0, 0), (1, 0)], padc,
                      lambda qc, ck: ck, ycT, masked=False)

            xr2 = act.tile([128, KC, TMY], F32R, tag="res", bufs=2, name="xr2")

            def cb_z2(p, m, w0, ww):
                z = tmp.tile([128, 512], F32, tag="t512", bufs=3, name=nm("z"))[:, :ww]
                nc.vector.tensor_scalar_add(z, p, bco8[:, m:m + 1])
                zk = tmp.tile([128, 512], F32, tag="t512", bufs=3, name=nm("zk"))[:, :ww]
                nc.vector.tensor_mul(zk, z, keepbc[:, w0:w0 + ww])
                nc.vector.tensor_add(xr2[:, m, w0:w0 + ww], zk, xr1[:, m, w0:w0 + ww])

            proj_fm(WCO, 0, KC, ycT, TMY, cb_z2)

            # ================ phase 5: MLP ================
            a3 = act.tile([128, KC, TMY], BF16, tag="bb", bufs=2, name="a3")
            layernorm(lambda k, nt: xr2[:, k, 512 * nt:512 * (nt + 1)],
                      TMY, ln2w, ln2b, a3)

            hT = act.tile([128, FKC, TMY], BF16, tag="hT", bufs=1)

            def cb_h(p, m, w0, ww):
                nc.scalar.activation(hT[:, m, w0:w0 + ww], p, GELU,
                                     bias=bfc32[:, m:m + 1])

            proj_fm(WFC, 0, FKC, a3, TMY, cb_h)

            outT = act.tile([128, KC, TMY], F32, tag="res", bufs=2, name="outT")
            for g in range(4):
                pouts = [ps.tile([128, 512], F32, tag="acc", bufs=2,
                                 name=nm("fp")) for _ in range(2)]
                for ksb in range(4):
                    slab = wslab(WFP, 256 * g, 256, kc0=8 * ksb, nk=8)
                    for i in range(2):
                        m = 2 * g + i
                        for k in range(8):
                            nc.tensor.matmul(
                                pouts[i], slab[:, k, 128 * i:128 * (i + 1)],
                                hT[:, 8 * ksb + k, :],
                                start=(ksb == 0 and k == 0),
                                stop=(ksb == 3 and k == 7))
                for i in range(2):
                    m = 2 * g + i
                    zf = tmp.tile([128, 512], F32, tag="t512", bufs=3,
                                  name=nm("zf"))
                    nc.vector.tensor_scalar_add(zf, pouts[i], bfp8[:, m:m + 1])
                    nc.vector.tensor_add(outT[:, m, :], zf, xr2[:, m, :])
            for m in range(KC):
                nc.sync.dma_start(out=OUT[128 * m:128 * (m + 1), :],
                                  in_=outT[:, m, :])

    nc.compile()
    return nc


def _host_prepare_general(core, inputs):
    import ml_dtypes
    BF = ml_dtypes.bfloat16
    b, half = core // 2, core % 2
    chunks = [0, 3, 1, 2] if half == 0 else [1, 2, 0, 3]
    idx = np.concatenate([np.arange(256 * c, 256 * (c + 1)) for c in chunks])

    x = np.asarray(inputs["x"], np.float32)
    pad = np.asarray(inputs["padding_mask"], bool)
    pocket = np.asarray(inputs["pocket_context"], np.float32)

    xT = np.ascontiguousarray(x[b][idx].T)
    pkT = np.ascontiguousarray(pocket[b].T).astype(BF)
    pad_loc = pad[b][idx]

    pads = np.zeros((128, 16), np.float32)
    attend = {0: {0, 1} | ({4, 5} if half == 1 else set()),
              1: {0, 1, 2, 3, 4, 5} | ({6, 7} if half == 0 else set())}
    for qc in range(2):
        for ck in range(8):
            col = 8 * qc + ck
            if ck not in attend[qc]:
                pads[:, col] = NEGB
            else:
                pads[:, col] = np.where(pad_loc[128 * ck:128 * (ck + 1)], NEGB, 0.0)
    pmask = np.abs(pocket[b]).sum(-1) == 0
    padc = np.zeros((128, 2), np.float32)
    for ck in range(2):
        padc[:, ck] = np.where(pmask[128 * ck:128 * (ck + 1)], NEGB, 0.0)
    keep = np.where(pad_loc[:TMY], 0.0, 1.0).astype(np.float32)

    t1 = np.where(np.arange(128)[None, :] >= np.arange(128)[:, None], 0.0,
                  NEG).astype(np.float32)
    m1 = np.concatenate([np.full((128, 128), NEG, np.float32), t1], axis=1)
    e2 = np.zeros((64, 128), np.float32)
    e2[0, 0:64] = 1.0
    e2[32, 64:128] = 1.0

    def pp(v):
        return np.ascontiguousarray(np.asarray(v, np.float32).reshape(-1, 128).T)

    def w(name):
        return np.asarray(inputs[name], np.float32).astype(BF)

    bqkv = np.asarray(inputs["bqkv"], np.float32)
    bkv = np.asarray(inputs["bkv"], np.float32)
    return {
        "XT": xT, "XMY": np.ascontiguousarray(xT[:, :TMY]), "PKT": pkT,
        "WQKV": w("Wqkv"), "WO": w("Wo"), "WQ": w("Wq"), "WKV": w("Wkv"),
        "WCO": w("Wco"), "WFC": w("Wfc"), "WFP": w("Wfp"),
        "BQK": pp(bqkv[:2048]), "BO8": pp(inputs["bo"]), "BQ8": pp(inputs["bq"]),
        "BKC8": pp(bkv[:1024]), "BCO8": pp(inputs["bco"]),
        "BFC32": pp(inputs["bfc"]), "BFP8": pp(inputs["bfp"]),
        "LN1W": pp(inputs["ln1_w"]), "LN1B": pp(inputs["ln1_b"]),
        "LNCW": pp(inputs["lnc_w"]), "LNCB": pp(inputs["lnc_b"]),
        "LN2W": pp(inputs["ln2_w"]), "LN2B": pp(inputs["ln2_b"]),
        "T1M": t1, "M1M": m1, "E2": e2.astype(BF),
        "ONL8": np.ones((128, 8), np.float32),
        "ON8B": np.full((8, 128), 0.125, np.float32),
        "EPS8": np.full((8, 1), EPS, np.float32),
        "ONES16": np.ones((128, 16), np.float32).astype(BF),
        "KEEPBC": np.broadcast_to(keep, (128, TMY)).copy(),
        "VBBC": np.broadcast_to(bqkv[2048:], (128, C)).astype(BF).copy(),
        "VCBBC": np.broadcast_to(bkv[1024:], (128, C)).astype(BF).copy(),
        "PADS": pads, "PADC": padc,
    }

